# revision 2
# baseline (speedup 1.0000x reference)
"""Trainium2 Bass kernel for nn_AttentionNet — v2.

Sharding: data-parallel over batch, 4 batches/core x 8 cores.

Encoder design (per batch):
  - src loaded NATURAL [tok, E] (fast DMA), LN1 computed along free axis on
    DVE (g/b folded into QKV weights host-side), then PE-transposed.
  - Scores in [keys, q] layout via fp8 DoubleRow matmuls; the -30*mask add
    rides the same PSUM accumulation as DoubleRow pairs (lhsT = mask slice,
    rhs = const [-30I|0 ; 0|-30I]) so masking costs no extra engine pass.
  - exp on ACT -> fp8 SBUF (eu8); AV + replicated-denominator matmuls in
    DoubleRow; softmax normalize = DVE reciprocal + multiply.
  - FFN w2 in DoubleRow fp8; LN2/dec-LN gains folded into w1 host-side.
Decoder + pointer: bf16, ACT Tanh; masked -10000 path kept exact in f32.
"""

import math

import numpy as np
import ml_dtypes

import concourse.bacc as bacc
import concourse.bass as bass
import concourse.tile as tile
from concourse import mybir
from concourse.bass_utils import run_bass_kernel_spmd

F32 = mybir.dt.float32
BF16 = mybir.dt.bfloat16
FP8 = mybir.dt.float8e4
I32 = mybir.dt.int32
AF = mybir.ActivationFunctionType
OP = mybir.AluOpType
PM = mybir.MatmulPerfMode

E, H, D, FF = 128, 4, 32, 512
HD = H * D
B, T, Q = 32, 1024, 1
NCORES = 8
BPC = B // NCORES
NKT = T // 128   # key tiles
NKP = NKT // 2   # key-tile pairs
SC = 1.0 / math.sqrt(D)
NMSK = -30.0     # mask additive constant (exp(-30*SC... no: exp(SC*(-30)) happens
                 # AFTER scale; we bake scale so exp sees SC*S - 30*SC*m... see below)

BF = ml_dtypes.bfloat16
F8 = ml_dtypes.float8_e4m3


def rep2(ap_):
    """Repeat a [P, F] AP twice along a new middle free dim via stride 0."""
    return bass.AP(tensor=ap_.tensor, offset=ap_.offset,
                   ap=[ap_.ap[0], [0, 2]] + list(ap_.ap[1:]))


def host_prep(inputs):
    """Fold LN gains/biases into weights; pre-cast/lay out weights."""
    inp = {k: np.asarray(v) for k, v in inputs.items()}
    d = {}
    for pfx in ("enc", "dec"):
        g1 = inp[f"{pfx}_ln1_g"].astype(np.float64)
        b1 = inp[f"{pfx}_ln1_b"].astype(np.float64)
        g2 = inp[f"{pfx}_ln2_g"].astype(np.float64)
        b2 = inp[f"{pfx}_ln2_b"].astype(np.float64)
        assert np.allclose(b1, 0) and np.allclose(b2, 0), "ln bias fold not implemented"
        for nm in ("wq", "wk", "wv"):
            w = inp[f"{pfx}_{nm}"].astype(np.float64)  # [H, E, D]
            w = w * g1[None, :, None]
            # -> [E, H*D]
            d[f"{pfx}_{nm}8"] = np.ascontiguousarray(
                w.transpose(1, 0, 2).reshape(E, HD)).astype(F8)
            d[f"{pfx}_{nm}16"] = np.ascontiguousarray(
                w.transpose(1, 0, 2).reshape(E, HD)).astype(BF)
        # wo: [H, D, E] -> [D, H, E] (lhsT tiles [32, h, E] at base 0)
        wo = inp[f"{pfx}_wo"].astype(np.float64).transpose(1, 0, 2)
        d[f"{pfx}_wo16"] = np.ascontiguousarray(wo).astype(BF)
        d[f"{pfx}_wo16f"] = np.ascontiguousarray(
            inp[f"{pfx}_wo"].astype(np.float64).reshape(HD, E)).astype(BF)
        # ffn w1 [E, FF] with g2 fold
        w1 = inp[f"{pfx}_ffn_w1"].astype(np.float64) * g2[:, None]
        d[f"{pfx}_w116"] = np.ascontiguousarray(w1).astype(BF)
        d[f"{pfx}_b1c"] = np.ascontiguousarray(
            inp[f"{pfx}_ffn_b1"].reshape(4, 128).T).astype(np.float32)  # [128, 4]
        # w2 [FF, E] -> DR-paired [128, pair 2, i 2, E]
        w2 = inp[f"{pfx}_ffn_w2"].astype(np.float64).reshape(2, 2, 128, E)
        d[f"{pfx}_w28"] = np.ascontiguousarray(w2.transpose(2, 0, 1, 3)).astype(F8)
        d[f"{pfx}_w216"] = np.ascontiguousarray(w2.transpose(2, 0, 1, 3)).astype(BF)
        d[f"{pfx}_b2c"] = np.ascontiguousarray(
            inp[f"{pfx}_ffn_b2"].reshape(E, 1)).astype(np.float32)
    d["ptr_wq16"] = inp["ptr_wq"].astype(BF)
    d["ptr_wk16"] = inp["ptr_wk"].astype(BF)

    # constants
    ipair = np.zeros((128, 2, 256), np.float32)
    for i in range(2):
        for r in range(128):
            ipair[r, i, 128 * i + r] = NMSK
    d["c_ipair"] = ipair.astype(F8)
    d["c_ident16"] = np.eye(128).astype(BF)
    d["c_ones8"] = np.ones((128, 2, 32), np.float32).astype(F8)
    d["c_onesrow"] = np.ones((1, 128), np.float32).astype(BF)
    # ind8 for partition-axis LN stats (batch b sums -> row b, sumsq -> 32+b)
    a = np.zeros((128, 8 * 36), np.float32)
    for b in range(4):
        a[:, 36 * b + b] = 1.0
        a[:, 36 * (4 + b) + 32 + b] = 1.0
    d["c_ind8"] = a.astype(BF)
    # ind2 for small partition-axis LN (sum -> row 0, sumsq -> row 32)
    a = np.zeros((128, 66), np.float32)
    a[:, 0] = 1.0
    a[:, 33 + 32] = 1.0
    d["c_ind2"] = a.astype(BF)
    d["c_eps4"] = np.full((4, 1), 1e-5, np.float32)
    d["c_eps128"] = np.full((128, 1), 1e-5, np.float32)
    d["c_negC"] = np.full((128, 1), -4.0, np.float32)
    d["c_eps1"] = np.full((1, 1), 1e-5, np.float32)
    # dsel[k, 4b+h] = 1 iff k == b  (decoder mask select)
    a = np.zeros((4, 16), np.float32)
    for b in range(BPC):
        a[b, 4 * b:4 * (b + 1)] = 1.0
    d["c_dsel"] = a.astype(BF)
    d["c_ones_col"] = np.ones((128, 1), np.float32).astype(BF)
    a = np.zeros((4, 4, 128), np.float32)
    for b in range(BPC):
        a[b, b, :] = 1.0
    d["c_rowsel"] = a.astype(BF)
    a = np.zeros((32, 4), np.float32)
    for i in range(32):
        a[i, i % 4] = 1.0
    d["c_p32"] = a.astype(np.float32)
    a = np.zeros((4, 128), np.float32)
    for h in range(H):
        a[h, 32 * h:32 * (h + 1)] = 1.0
    d["c_e4t"] = a.astype(np.float32)
    return d


DERIVED_SPECS = None  # filled on first host_prep


def _emit(nc, tc, tens, ctx):
    import os
    _dbg = bool(os.environ.get("KDBG"))
    _kph = int(os.environ.get("KPH", "9"))
    singles = ctx.enter_context(tc.tile_pool(name="singles", bufs=1))
    psum = ctx.enter_context(tc.tile_pool(name="psum", bufs=1, space="PSUM"))
    big = ctx.enter_context(tc.tile_pool(name="big", bufs=1))
    scr = ctx.enter_context(tc.tile_pool(name="scr", bufs=1))

    cnt = [0]

    def ps_S():  # [128, 1024] f32: scores / V-proj / srcT staging (2 banks)
        cnt[0] += 1
        return psum.tile([128, 1024], F32, tag="S", name=f"S{cnt[0]}", bufs=1)

    def ps_T():  # [128, 1024] bf16 transposes (1 bank)
        cnt[0] += 1
        return psum.tile([128, 1024], BF16, tag="T", name=f"T{cnt[0]}", bufs=1)

    def ps_a():  # [32, 512] f32 AV (1 bank)
        cnt[0] += 1
        return psum.tile([32, 512], F32, tag="a", name=f"a{cnt[0]}", bufs=1)

    def ps_d():  # [32, 512] f32 denom (1 bank)
        cnt[0] += 1
        return psum.tile([32, 512], F32, tag="d", name=f"d{cnt[0]}", bufs=2)

    def ps_G():  # [128, 512] f32 general (1 bank)
        cnt[0] += 1
        return psum.tile([128, 512], F32, tag="G", name=f"G{cnt[0]}", bufs=2)

    def load(name, shape, dt, ap=None):
        tl = singles.tile(shape, dt, tag=name, name=name)
        src_ap = tens[name].ap() if ap is None else ap
        if dt in (FP8, BF16) and src_ap.dtype != dt:
            src_ap = src_ap.bitcast(dt)
        nc.sync.dma_start(out=tl[:], in_=src_ap)
        return tl

    # ---- weights / constants to SBUF ----
    w = {}
    for pfx in ("enc", "dec"):
        for nm in ("wq8", "wk8", "wv8"):
            w[f"{pfx}_{nm}"] = load(f"{pfx}_{nm}", [E, HD], FP8)
        for nm in ("wq16", "wk16", "wv16"):
            w[f"{pfx}_{nm}"] = load(f"{pfx}_{nm}", [E, HD], BF16)
        w[f"{pfx}_wo16"] = load(f"{pfx}_wo16", [D, H, E], BF16)
        w[f"{pfx}_wo16f"] = load(f"{pfx}_wo16f", [HD, E], BF16)
        w[f"{pfx}_w116"] = load(f"{pfx}_w116", [E, FF], BF16)
        w[f"{pfx}_b1c"] = load(f"{pfx}_b1c", [128, 4], F32)
        w[f"{pfx}_w28"] = load(f"{pfx}_w28", [128, 2, 2, E], FP8)
        w[f"{pfx}_w216"] = load(f"{pfx}_w216", [128, 2, 2, E], BF16)
        w[f"{pfx}_b2c"] = load(f"{pfx}_b2c", [E, 1], F32)
    ptrq = load("ptr_wq16", [E, E], BF16)
    ptrk = load("ptr_wk16", [E, E], BF16)
    ipair = load("c_ipair", [128, 2, 256], FP8)
    ident = load("c_ident16", [128, 128], BF16)
    ones8 = load("c_ones8", [128, 2, 32], FP8)
    onesrow = load("c_onesrow", [1, 128], BF16)
    ind8 = load("c_ind8", [128, 288], BF16)
    ind2 = load("c_ind2", [128, 66], BF16)
    eps4 = load("c_eps4", [4, 1], F32)
    eps128 = load("c_eps128", [128, 1], F32)
    negC = load("c_negC", [128, 1], F32)
    eps1 = load("c_eps1", [1, 1], F32)
    dsel = load("c_dsel", [4, 16], BF16)
    ones_col = load("c_ones_col", [128, 1], BF16)
    p32 = load("c_p32", [32, 4], F32)
    rowsel = load("c_rowsel", [4, 4, 128], BF16)
    e4t = load("c_e4t", [4, 128], F32)

    # persistent per-batch outputs
    srcTb = [big.tile([E, T], BF16, tag=f"srcT{b}", name=f"srcT{b}") for b in range(BPC)]
    h1b = [big.tile([E, T], BF16, tag=f"h1{b}", name=f"h1{b}") for b in range(BPC)]
    memb = [big.tile([E, T], BF16, tag=f"mem{b}", name=f"memb{b}") for b in range(BPC)]

    # ---------------- encoder: per batch ----------------
    for b in range(BPC):
        with nc.named_scope(f"enc_b{b}"):
            # src natural [128, 8, 128] f32 (one DMA)
            xsrc = scr.tile([128, NKT, E], F32, tag="xsrc", bufs=2)
            nc.sync.dma_start(out=xsrc[:],
                              in_=tens["src"].ap()[b].rearrange("(c p) e -> p c e", p=128))
            # masks: 2 DMAs of [128, 4, 1024] i32, cast to fp8 on Pool
            m8 = scr.tile([128, NKT, T], FP8, tag="m8", bufs=1 if _dbg else 2)
            for quar in range(4):
                mi = scr.tile([128, 2, T], I32, tag="mi", bufs=2)
                nc.sync.dma_start(
                    out=mi[:],
                    in_=tens["enc_mask"].ap()[b].rearrange(
                        "(c p) t -> p c t", p=128)[:, 2 * quar:2 * (quar + 1), :])
                nc.gpsimd.tensor_copy(m8[:, 2 * quar:2 * (quar + 1), :], mi[:])

            # LN1 along free axis (per 128-token tile)
            ssum = scr.tile([128, NKT], F32, tag="ssum")
            ssq = scr.tile([128, NKT], F32, tag="ssq")
            sqscr = scr.tile([128, E], F32, tag="sqscr")
            for t in range(NKT):
                nc.vector.tensor_reduce(ssum[:, t:t + 1], xsrc[:, t, :],
                                        axis=mybir.AxisListType.X, op=OP.add)
                nc.vector.tensor_mul(sqscr[:], xsrc[:, t, :], xsrc[:, t, :])
                nc.vector.tensor_reduce(ssq[:, t:t + 1], sqscr[:],
                                        axis=mybir.AxisListType.X, op=OP.add)
            mcol = scr.tile([128, NKT], F32, tag="mcol")
            nc.vector.tensor_scalar_mul(mcol[:], ssum[:], 1.0 / E)
            var = scr.tile([128, NKT], F32, tag="var")
            nc.vector.scalar_tensor_tensor(out=var[:], in0=ssq[:], scalar=1.0 / E,
                                           in1=mcol[:], op0=OP.mult, op1=OP.bypass)
            msq = scr.tile([128, NKT], F32, tag="msq")
            nc.vector.tensor_mul(msq[:], mcol[:], mcol[:])
            nc.vector.tensor_sub(var[:], var[:], msq[:])
            sd = scr.tile([128, NKT], F32, tag="sd")
            nc.scalar.activation(out=sd[:], in_=var[:], func=AF.Sqrt, bias=eps128[:, 0:1])
            rs = scr.tile([128, NKT], F32, tag="rs")
            nc.vector.reciprocal(rs[:], sd[:])
            # xln bf16 tiles + PE transpose into XLT (bf16 psum), srcT (bf16)
            xln = scr.tile([128, NKT, E], BF16, tag="xln")
            for t in range(NKT):
                nc.vector.tensor_scalar(out=xln[:, t, :], in0=xsrc[:, t, :],
                                        scalar1=mcol[:, t:t + 1], scalar2=rs[:, t:t + 1],
                                        op0=OP.subtract, op1=OP.mult)
            xlt_ps = ps_T()
            for t in range(NKT):
                nc.tensor.matmul(xlt_ps[:, 128 * t:128 * (t + 1)], xln[:, t, :],
                                 ident[:], start=True, stop=True, is_transpose=True)
            xlt8 = scr.tile([E, T], FP8, tag="xlt8")
            nc.vector.tensor_copy(xlt8[:], xlt_ps[:])
            # src transpose for residual (bf16)
            xsb = scr.tile([128, NKT, E], BF16, tag="xsb")
            nc.vector.tensor_copy(xsb[:], xsrc[:])
            st_ps = ps_T()
            for t in range(NKT):
                nc.tensor.matmul(st_ps[:, 128 * t:128 * (t + 1)], xsb[:, t, :],
                                 ident[:], start=True, stop=True, is_transpose=True)
            nc.vector.tensor_copy(srcTb[b][:], st_ps[:])

            # ---- QKV (head-pair split so lhsT/rhs bases land on 0/32) ----
            # q8z: [64, hp 2, qp 4, i 2, 256] zero-interleaved, SC prefolded
            q8z = scr.tile([64, 2, 4, 2, 256], FP8, tag="q8z")
            nc.vector.memset(q8z[:], 0.0)
            # k8h: [64, hp 2, T]
            k8h = scr.tile([64, 2, T], FP8, tag="k8h")
            for c in range(2):
                s = slice(512 * c, 512 * (c + 1))
                for hp in range(2):
                    hs = slice(64 * hp, 64 * (hp + 1))
                    qp = ps_G()
                    nc.tensor.matmul(qp[0:64, :], w["enc_wq8"][:, hs], xlt8[:, s],
                                     start=True, stop=True)
                    base = q8z[:].rearrange("p a b i f -> p (a b i f)")
                    dst = bass.AP(tensor=base.tensor,
                                  offset=base.offset + 2048 * hp + 1024 * c,
                                  ap=[base.ap[0], [512, 2], [384, 2], [1, 128]])
                    nc.vector.tensor_scalar_mul(
                        dst, qp[0:64, :].rearrange("p (a i f) -> p a i f", a=2, i=2), SC)
                    kp_ = ps_G()
                    nc.tensor.matmul(kp_[0:64, :], w["enc_wk8"][:, hs], xlt8[:, s],
                                     start=True, stop=True)
                    nc.vector.tensor_copy(k8h[:, hp, s], kp_[0:64, :])
            # V natural (+ ones cols prefilled)
            v8 = scr.tile([128, NKP, 2, H, 64], FP8, tag="v8")
            nc.vector.memset(v8[:], 1.0)
            vp = ps_S()
            for t in range(NKT):
                nc.tensor.matmul(vp[:, 128 * t:128 * (t + 1)],
                                 xlt8[:, 128 * t:128 * (t + 1)], w["enc_wv8"][:],
                                 start=True, stop=True)
            vsrc = vp[:].rearrange("p (kp i h d) -> p kp i h d", kp=NKP, i=2, h=H)
            nc.vector.tensor_copy(v8[:, :, :, :, 0:32], vsrc)

            if _kph < 2:
                continue
            # ---- scores + mask + exp ----
            eu8 = scr.tile([128, NKP, 2, H, T], FP8, tag="eu8", bufs=1)
            for h in range(H):
                hb = slice(32 * (h % 2), 32 * (h % 2) + 32)
                krow = k8h[:, h // 2, :]
                for kt in range(NKT):
                    Sp = ps_S()
                    for qp_i in range(4):
                        qs = slice(256 * qp_i, 256 * (qp_i + 1))
                        nc.tensor.matmul(Sp[:, qs],
                                         rep2(krow[hb, 128 * kt:128 * (kt + 1)]),
                                         q8z[hb, h // 2, qp_i, :, :],
                                         start=True, stop=False, perf_mode=PM.DoubleRow)
                        nc.tensor.matmul(Sp[:, qs],
                                         m8[:, 2 * qp_i:2 * qp_i + 2,
                                            128 * kt:128 * (kt + 1)],
                                         ipair[:], start=False, stop=True,
                                         perf_mode=PM.DoubleRow)
                    nc.scalar.activation(out=eu8[:, kt // 2, kt % 2, h, :], in_=Sp[:],
                                         func=AF.Exp, bias=negC[:, 0:1])
                    if b == 0 and h == 0 and kt == 0 and "dbg_S" in tens:
                        Ssb = scr.tile([128, T], F32, tag="dbg")
                        nc.vector.tensor_copy(Ssb[:], Sp[:])
                        nc.sync.dma_start(out=tens["dbg_S"].ap(), in_=Ssb[:])
            # NOTE: mask adds NMSK (not NMSK*SC) because q8z is pre-scaled by SC
            # and the exp has scale=1 -> exp(SC*K.Q + NMSK*m). exp(-30) ~ 1e-13.

            if _kph < 3:
                continue
            # ---- AV + denom + normalize + wo ----
            for c in range(2):
                s = slice(512 * c, 512 * (c + 1))
                h1p = ps_G()
                for h in range(H):
                    av = ps_a()
                    dn = ps_d()
                    for kp_i in range(NKP):
                        nc.tensor.matmul(av[:], v8[:, kp_i, :, h, 0:32],
                                         eu8[:, kp_i, :, h, s],
                                         start=(kp_i == 0), stop=(kp_i == NKP - 1),
                                         perf_mode=PM.DoubleRow)
                        nc.tensor.matmul(dn[:], ones8[:],
                                         eu8[:, kp_i, :, h, s],
                                         start=(kp_i == 0), stop=(kp_i == NKP - 1),
                                         perf_mode=PM.DoubleRow)
                    rc = scr.tile([32, 512], F32, tag="rc")
                    nc.vector.reciprocal(rc[:], dn[:])
                    hn = scr.tile([32, 512], BF16, tag="hn")
                    nc.vector.tensor_tensor(out=hn[:], in0=av[:], in1=rc[:], op=OP.mult)
                    if b == 0 and h == 0 and c == 0 and "dbg_dn" in tens:
                        dsb = scr.tile([32, 512], F32, tag="dbg2")
                        nc.vector.tensor_copy(dsb[:], dn[:])
                        nc.sync.dma_start(out=tens["dbg_dn"].ap(), in_=dsb[:])
                        asb = scr.tile([32, 512], F32, tag="dbg2")
                        nc.vector.tensor_copy(asb[:], av[:])
                        nc.sync.dma_start(out=tens["dbg_av"].ap(), in_=asb[:])
                    nc.tensor.matmul(h1p[:], w["enc_wo16"][:, h, :], hn[:],
                                     start=(h == 0), stop=(h == H - 1))
                nc.vector.tensor_add(h1b[b][:, s], h1p[:], srcTb[b][:, s])
            if b == 0 and "dbg_h1" in tens:
                h1f = scr.tile([128, T], F32, tag="dbg")
                nc.vector.tensor_copy(h1f[:], h1b[0][:])
                nc.sync.dma_start(out=tens["dbg_h1"].ap(), in_=h1f[:])
                x8f = scr.tile([128, T], F32, tag="dbg")
                nc.vector.tensor_copy(x8f[:], xlt8[:])
                nc.sync.dma_start(out=tens["dbg_xlt"].ap(), in_=x8f[:])

    if _kph < 5:
        stub = scr.tile([BPC, T], F32, tag="lg_m", name="stub", bufs=1)
        hsrc = h1b[0] if _kph >= 3 else srcTb[0]
        nc.vector.tensor_copy(stub[:], hsrc[0:BPC, :])
        nc.sync.dma_start(out=tens["out"].ap().rearrange("b q t -> (b q) t"), in_=stub[:])
        return

    # ---------------- encoder LN2 + FFN (all batches) ----------------
    def ln_stats_group(xs, tagn):
        """Partition-axis LN core stats for 4 bf16 [E, T] tiles.
        Returns (mrow, nmrow, rsrow): [4, T] f32/bf16 tiles (per-batch rows)."""
        stats = ps_S()
        for b, xt in enumerate(xs):
            sq = scr.tile([128, T], BF16, tag="lg_sq", name=f"sq{tagn}", bufs=2)
            nc.vector.tensor_mul(sq[:], xt[:], xt[:])
            for c in range(2):
                s = slice(512 * c, 512 * (c + 1))
                nc.tensor.matmul(stats[0:36, s], ind8[:, 36 * b:36 * (b + 1)],
                                 xt[:, s], start=(b == 0), stop=False)
                nc.tensor.matmul(stats[0:36, s], ind8[:, 36 * (4 + b):36 * (5 + b)],
                                 sq[:, s], start=False,
                                 stop=(b == len(xs) - 1))
        m = scr.tile([4, T], F32, tag="lg_m", name=f"m{tagn}")
        nc.vector.tensor_scalar_mul(m[:], stats[0:4, :], 1.0 / E)
        var = scr.tile([4, T], F32, tag="lg_v", name=f"v{tagn}")
        nc.vector.tensor_scalar_mul(var[:], stats[32:36, :], 1.0 / E)
        msq = scr.tile([4, T], F32, tag="lg_ms", name=f"ms{tagn}")
        nc.vector.tensor_mul(msq[:], m[:], m[:])
        nc.vector.tensor_sub(var[:], var[:], msq[:])
        sd = scr.tile([4, T], F32, tag="lg_sd", name=f"sd{tagn}")
        nc.scalar.activation(out=sd[:], in_=var[:], func=AF.Sqrt, bias=eps4[:, 0:1])
        rsf = scr.tile([4, T], F32, tag="lg_rf", name=f"rsf{tagn}")
        nc.vector.reciprocal(rsf[:], sd[:])
        rs = scr.tile([4, T], BF16, tag="lg_rs", name=f"rs{tagn}", bufs=2)
        nc.vector.tensor_copy(rs[:], rsf[:])
        nm = scr.tile([4, T], BF16, tag="lg_nm", name=f"nm{tagn}", bufs=2)
        nc.vector.scalar_tensor_tensor(out=nm[:], in0=m[:], scalar=-1.0,
                                       in1=rsf[:], op0=OP.mult, op1=OP.mult)
        return rs, nm

    def ln_apply(xt, rs, nm, b, c, out, out_slice=None):
        """out[:, s] = x*(ones x rs_b) + (ones x nm_b) for 512-chunk c."""
        s = slice(512 * c, 512 * (c + 1))
        a_ps = ps_G()
        nc.tensor.matmul(a_ps[:], rowsel[:, b, :], rs[0:4, s], start=True, stop=True)
        b_ps = ps_G()
        nc.tensor.matmul(b_ps[:], rowsel[:, b, :], nm[0:4, s], start=True, stop=True)
        tmp = scr.tile([128, 512], F32, tag="lntmp")
        nc.vector.scalar_tensor_tensor(out=tmp[:], in0=xt[:, s], scalar=1.0,
                                       in1=a_ps[:], op0=OP.bypass, op1=OP.mult)
        dst = out[:, s] if out_slice is None else out_slice
        nc.vector.scalar_tensor_tensor(out=dst, in0=tmp[:], scalar=1.0,
                                       in1=b_ps[:], op0=OP.bypass, op1=OP.add)

    with nc.named_scope("enc_ffn"):
        rs2, nm2 = ln_stats_group(h1b, "l2")
        for b in range(BPC):
            act8 = scr.tile([128, 2, 2, T], FP8, tag="act8")
            hln = scr.tile([E, T], BF16, tag="hln")
            for c in range(2):
                ln_apply(h1b[b], rs2, nm2, b, c, hln)
            for fc in range(4):
                for c in range(2):
                    s = slice(512 * c, 512 * (c + 1))
                    g = ps_G()
                    nc.tensor.matmul(g[:], w["enc_w116"][:, 128 * fc:128 * (fc + 1)],
                                     hln[:, s], start=True, stop=True)
                    nc.vector.tensor_scalar(out=act8[:, fc // 2, fc % 2, s], in0=g[:],
                                            scalar1=w["enc_b1c"][:, fc:fc + 1],
                                            scalar2=0.0, op0=OP.add, op1=OP.max)
            for c in range(2):
                s = slice(512 * c, 512 * (c + 1))
                mp = ps_G()
                for p in range(2):
                    nc.tensor.matmul(mp[:], w["enc_w28"][:, p, :, :],
                                     act8[:, p, :, s], start=(p == 0), stop=(p == 1),
                                     perf_mode=PM.DoubleRow)
                tmp2 = scr.tile([128, 512], F32, tag="ffntmp")
                nc.vector.tensor_scalar(out=tmp2[:], in0=mp[:],
                                        scalar1=w["enc_b2c"][:, 0:1], scalar2=None,
                                        op0=OP.add)
                nc.vector.tensor_add(memb[b][:, s], tmp2[:], h1b[b][:, s])

    if _kph < 6:
        stub = scr.tile([BPC, T], F32, tag="lg_m", name="stub", bufs=1)
        nc.vector.tensor_copy(stub[:], memb[0][0:BPC, :])
        nc.sync.dma_start(out=tens["out"].ap().rearrange("b q t -> (b q) t"), in_=stub[:])
        return

    # ---------------- decoder ----------------
    with nc.named_scope("decoder"):
        tgtT = singles.tile([E, BPC], F32, tag="tgtT")
        nc.sync.dma_start(out=tgtT[:], in_=tens["tgt"].ap().rearrange("b q e -> e (b q)"))
        dmi = scr.tile([BPC, T], I32, tag="dmi", bufs=1)
        nc.sync.dma_start(out=dmi[:], in_=tens["dec_mask"].ap().rearrange("b q t -> (b q) t"))
        dmf = singles.tile([BPC, T], BF16, tag="dmf")
        nc.vector.tensor_scalar_mul(dmf[:], dmi[:], -10000.0)
        dmf32 = singles.tile([BPC, T], F32, tag="dmf32")
        nc.vector.tensor_scalar_mul(dmf32[:], dmi[:], -10000.0)
        dwf = singles.tile([BPC, T], F32, tag="dwf")
        nc.vector.tensor_scalar(out=dwf[:], in0=dmi[:], scalar1=-1.0, scalar2=1.0,
                                op0=OP.mult, op1=OP.add)

        def ln_small(x, n, tagn):
            """Core LN over partitions for [E, n] f32 -> bf16 (g/b folded away)."""
            xbf = scr.tile([E, BPC], BF16, tag=f"dx{tagn}")
            nc.vector.tensor_copy(xbf[:, :n], x[:, :n])
            sq = scr.tile([E, BPC], BF16, tag=f"dq{tagn}")
            nc.vector.tensor_mul(sq[:, :n], xbf[:, :n], xbf[:, :n])
            stp = ps_G()
            nc.tensor.matmul(stp[0:33, :n], ind2[:, 0:33], xbf[:, :n], start=True, stop=False)
            nc.tensor.matmul(stp[0:33, :n], ind2[:, 33:66], sq[:, :n], start=False, stop=True)
            st = scr.tile([33, BPC], F32, tag=f"ds{tagn}")
            nc.vector.tensor_copy(st[:, :n], stp[0:33, :n])
            mn = scr.tile([1, BPC], F32, tag=f"dm{tagn}")
            nc.vector.tensor_scalar_mul(mn[:, :n], st[0:1, :n], 1.0 / E)
            msq = scr.tile([1, BPC], F32, tag=f"dmq{tagn}")
            nc.vector.tensor_mul(msq[:, :n], mn[:, :n], mn[:, :n])
            var = scr.tile([1, BPC], F32, tag=f"dv{tagn}")
            nc.vector.tensor_scalar_mul(var[:, :n], st[32:33, :n], 1.0 / E)
            nc.vector.tensor_sub(var[:, :n], var[:, :n], msq[:, :n])
            sdd = scr.tile([1, BPC], F32, tag=f"dsd{tagn}")
            nc.scalar.activation(out=sdd[:, :n], in_=var[:, :n], func=AF.Sqrt,
                                 bias=eps1[:, 0:1])
            rsf = scr.tile([1, BPC], F32, tag=f"drf{tagn}")
            nc.vector.reciprocal(rsf[:, :n], sdd[:, :n])
            rs = scr.tile([1, BPC], BF16, tag=f"dr{tagn}")
            nc.vector.tensor_copy(rs[:, :n], rsf[:, :n])
            nm = scr.tile([1, BPC], BF16, tag=f"dn{tagn}")
            nc.vector.scalar_tensor_tensor(out=nm[:, :n], in0=mn[:, :n], scalar=-1.0,
                                           in1=rsf[:, :n], op0=OP.mult, op1=OP.mult)
            a_ps = ps_G()
            nc.tensor.matmul(a_ps[:, 0:n], onesrow[:], rs[:, :n], start=True, stop=True)
            b_ps = ps_G()
            nc.tensor.matmul(b_ps[:, 0:n], onesrow[:], nm[:, :n], start=True, stop=True)
            tmp = scr.tile([E, BPC], F32, tag=f"dt{tagn}")
            nc.vector.scalar_tensor_tensor(out=tmp[:, :n], in0=x[:, :n], scalar=1.0,
                                           in1=a_ps[:, 0:n], op0=OP.bypass, op1=OP.mult)
            out = scr.tile([E, BPC], BF16, tag=f"do{tagn}")
            nc.vector.scalar_tensor_tensor(out=out[:, :n], in0=tmp[:, :n], scalar=1.0,
                                           in1=b_ps[:, 0:n], op0=OP.bypass, op1=OP.add)
            return out

        tln = ln_small(tgtT, BPC, "t")
        qd_ps = ps_G()
        nc.tensor.matmul(qd_ps[0:HD, 0:BPC], w["dec_wq16"][:], tln[:, :BPC],
                         start=True, stop=True)
        qdec = scr.tile([HD, BPC], BF16, tag="qdec")
        nc.vector.tensor_copy(qdec[:], qd_ps[0:HD, 0:BPC])

        rsm, nmm = ln_stats_group(memb, "lm")
        h1d = singles.tile([E, BPC], F32, tag="h1d")
        for b in range(BPC):
            mln = scr.tile([E, T], BF16, tag="mln")
            for c in range(2):
                ln_apply(memb[b], rsm, nmm, b, c, mln)
            kd = scr.tile([HD, T], BF16, tag="kd", bufs=1)
            for c in range(2):
                s = slice(512 * c, 512 * (c + 1))
                kp_ = ps_G()
                nc.tensor.matmul(kp_[:], w["dec_wk16"][:], mln[:, s], start=True, stop=True)
                nc.vector.tensor_copy(kd[:, s], kp_[:])
            vd = scr.tile([128, NKT, HD], BF16, tag="vd", bufs=1)
            vp2 = ps_S()
            for t in range(NKT):
                nc.tensor.matmul(vp2[:, 128 * t:128 * (t + 1)],
                                 mln[:, 128 * t:128 * (t + 1)], w["dec_wv16"][:],
                                 start=True, stop=True)
            nc.vector.tensor_copy(vd[:], vp2[:].rearrange("p (c e) -> p c e", c=NKT))
            qblk = scr.tile([HD, 4], BF16, tag="qblk")
            nc.vector.memset(qblk[:], 0.0)
            for h in range(H):
                nc.vector.tensor_copy(qblk[32 * h:32 * (h + 1), h:h + 1],
                                      qdec[32 * h:32 * (h + 1), b:b + 1])
            ud_ps = ps_G()
            for kt in range(NKT):
                cs = slice(4 * kt, 4 * (kt + 1))
                nc.tensor.matmul(ud_ps[:, cs], kd[:, 128 * kt:128 * (kt + 1)], qblk[:],
                                 start=True, stop=False)
                nc.tensor.matmul(ud_ps[:, cs], dmf[0:4, 128 * kt:128 * (kt + 1)],
                                 dsel[:, 4 * b:4 * (b + 1)], start=False, stop=True)
            eud = scr.tile([128, 4 * NKT], BF16, tag="eud")
            nc.scalar.activation(out=eud[:], in_=ud_ps[:, 0:4 * NKT], func=AF.Exp,
                                 scale=SC)
            d1_ps = ps_G()
            nc.tensor.matmul(d1_ps[0:32, 0:1], eud[:], ones_col[:], start=True, stop=True)
            d1 = scr.tile([32, 1], F32, tag="d1s")
            nc.vector.tensor_copy(d1[:], d1_ps[0:32, 0:1])
            d4_ps = ps_G()
            nc.tensor.matmul(d4_ps[0:4, 0:1], p32[:], d1[:], start=True, stop=True)
            rc4 = scr.tile([4, 1], F32, tag="rc4")
            nc.vector.reciprocal(rc4[:], d4_ps[0:4, 0:1])
            rb_ps = ps_G()
            nc.tensor.matmul(rb_ps[:, 0:1], e4t[:], rc4[:], start=True, stop=True)
            rb = scr.tile([128, 1], F32, tag="rb")
            nc.vector.tensor_copy(rb[:], rb_ps[:, 0:1])
            hd_ps = ps_G()
            for kt in range(NKT):
                nc.tensor.matmul(hd_ps[:, 0:4], vd[:, kt, :], eud[:, 4 * kt:4 * (kt + 1)],
                                 start=(kt == 0), stop=(kt == NKT - 1))
            hdec = scr.tile([HD, 1], BF16, tag="hdec")
            for h in range(H):
                nc.vector.tensor_copy(hdec[32 * h:32 * (h + 1), 0:1],
                                      hd_ps[32 * h:32 * (h + 1), h:h + 1])
            nc.vector.tensor_scalar_mul(hdec[:], hdec[:], rb[:, 0:1])
            ao_ps = ps_G()
            nc.tensor.matmul(ao_ps[:, 0:1], w["dec_wo16f"][:], hdec[:],
                             start=True, stop=True)
            nc.vector.tensor_add(h1d[:, b:b + 1], ao_ps[:, 0:1], tgtT[:, b:b + 1])

        hln2d = ln_small(h1d, BPC, "d2")
        dact_ps = ps_G()
        for fc in range(4):
            nc.tensor.matmul(dact_ps[:, 4 * fc:4 * (fc + 1)],
                             w["dec_w116"][:, 128 * fc:128 * (fc + 1)], hln2d[:, :BPC],
                             start=True, stop=True)
        dact = scr.tile([128, 2, 2, 4], FP8, tag="dact")
        for fc in range(4):
            nc.vector.tensor_scalar(out=dact[:, fc // 2, fc % 2, :],
                                    in0=dact_ps[:, 4 * fc:4 * (fc + 1)],
                                    scalar1=w["dec_b1c"][:, fc:fc + 1],
                                    scalar2=0.0, op0=OP.add, op1=OP.max)
        do_ps = ps_G()
        for p in range(2):
            nc.tensor.matmul(do_ps[:, 0:BPC], w["dec_w28"][:, p, :, :],
                             dact[:, p, :, :], start=(p == 0), stop=(p == 1),
                             perf_mode=PM.DoubleRow)
        decT = singles.tile([E, BPC], F32, tag="decT")
        tmp2 = scr.tile([E, BPC], F32, tag="dtmp2")
        nc.vector.tensor_scalar(out=tmp2[:], in0=do_ps[:, 0:BPC],
                                scalar1=w["dec_b2c"][:, 0:1], scalar2=None, op0=OP.add)
        nc.vector.tensor_add(decT[:], tmp2[:], h1d[:])

    if _kph < 7:
        stub = scr.tile([BPC, T], F32, tag="lg_m", name="stub", bufs=1)
        nc.vector.tensor_copy(stub[:], memb[0][0:BPC, :])
        nc.sync.dma_start(out=tens["out"].ap().rearrange("b q t -> (b q) t"), in_=stub[:])
        return

    # ---------------- pointer ----------------
    with nc.named_scope("pointer"):
        dec16 = scr.tile([E, BPC], BF16, tag="dec16")
        nc.vector.tensor_copy(dec16[:], decT[:])
        qp_ps = ps_G()
        nc.tensor.matmul(qp_ps[:, 0:BPC], ptrq[:], dec16[:], start=True, stop=True)
        qpi = scr.tile([E, 4 * BPC], BF16, tag="qpi")
        nc.vector.memset(qpi[:], 0.0)
        for b in range(BPC):
            nc.vector.tensor_copy(qpi[:, 5 * b:5 * b + 1], qp_ps[:, b:b + 1])
        up_ps = ps_S()
        for b in range(BPC):
            kp8 = scr.tile([E, T], BF16, tag="kp8", bufs=1)
            for c in range(2):
                s = slice(512 * c, 512 * (c + 1))
                kpc = ps_G()
                nc.tensor.matmul(kpc[:], ptrk[:], memb[b][:, s], start=True, stop=True)
                nc.vector.tensor_copy(kp8[:, s], kpc[:])
            for c in range(2):
                s = slice(512 * c, 512 * (c + 1))
                nc.tensor.matmul(up_ps[0:BPC, s], qpi[:, 4 * b:4 * (b + 1)], kp8[:, s],
                                 start=(b == 0), stop=(b == BPC - 1))
        # L = 10*tanh(U/sqrt(E)); masked -> *dwf + dmf32; log_softmax
        th = scr.tile([BPC, T], F32, tag="th", bufs=1)
        nc.scalar.activation(out=th[:], in_=up_ps[0:BPC, :], func=AF.Tanh,
                             scale=1.0 / math.sqrt(E))
        L = scr.tile([BPC, T], F32, tag="L", bufs=1)
        nc.vector.scalar_tensor_tensor(out=L[:], in0=th[:], scalar=10.0,
                                       in1=dwf[:], op0=OP.mult, op1=OP.mult)
        nc.vector.tensor_add(L[:], L[:], dmf32[:])
        et = scr.tile([BPC, T], F32, tag="lg_ms", name="et", bufs=1)
        se = scr.tile([BPC, 1], F32, tag="se")
        nc.scalar.activation(out=et[:], in_=L[:], func=AF.Exp, accum_out=se[:])
        lse = scr.tile([BPC, 1], F32, tag="lse")
        nc.scalar.activation(out=lse[:], in_=se[:], func=AF.Ln)
        res = scr.tile([BPC, T], F32, tag="lg_m", name="res", bufs=1)
        nc.vector.tensor_scalar(out=res[:], in0=L[:], scalar1=lse[:, 0:1], scalar2=None,
                                op0=OP.subtract)
        nc.sync.dma_start(out=tens["out"].ap().rearrange("b q t -> (b q) t"), in_=res[:])


def build():
    import contextlib
    global DERIVED_SPECS
    nc = bacc.Bacc()
    tens = {}
    tens["src"] = nc.dram_tensor("src", [BPC, T, E], F32, kind="ExternalInput")
    tens["tgt"] = nc.dram_tensor("tgt", [BPC, Q, E], F32, kind="ExternalInput")
    tens["enc_mask"] = nc.dram_tensor("enc_mask", [BPC, T, T], I32, kind="ExternalInput")
    tens["dec_mask"] = nc.dram_tensor("dec_mask", [BPC, Q, T], I32, kind="ExternalInput")
    for name, arr in DERIVED_SPECS.items():
        if arr.dtype == F8:
            dt_ = mybir.dt.uint8
        elif arr.dtype == BF:
            dt_ = mybir.dt.uint16
        else:
            dt_ = mybir.dt.from_np(arr.dtype)
        tens[name] = nc.dram_tensor(name, list(arr.shape), dt_, kind="ExternalInput")
    tens["out"] = nc.dram_tensor("out", [BPC, Q, T], F32, kind="ExternalOutput")
    import os
    if os.environ.get("KDBG"):
        for nm, shp in [("dbg_S", [128, T]), ("dbg_dn", [32, 512]),
                        ("dbg_av", [32, 512]), ("dbg_h1", [128, T]),
                        ("dbg_xlt", [128, T])]:
            tens[nm] = nc.dram_tensor(nm, shp, F32, kind="ExternalOutput")

    with tile.TileContext(nc) as tc:
        with contextlib.ExitStack() as ctx:
            _emit(nc, tc, tens, ctx)
    nc.finalize()
    return nc


_built = {}


def _get_nc():
    if "nc" not in _built:
        _built["nc"] = build()
    return _built["nc"]


def make_in_maps(inputs, derived):
    in_maps = []
    for c in range(NCORES):
        s = slice(BPC * c, BPC * (c + 1))
        m = {
            "src": np.ascontiguousarray(np.asarray(inputs["src"])[s]),
            "tgt": np.ascontiguousarray(np.asarray(inputs["tgt"])[s]),
            "enc_mask": np.ascontiguousarray(np.asarray(inputs["enc_mask"])[s]),
            "dec_mask": np.ascontiguousarray(np.asarray(inputs["dec_mask"])[s]),
        }
        m.update(_wire_dtypes(derived))
        in_maps.append(m)
    return in_maps


def _wire_dtypes(derived):
    """fp8/bf16 arrays cross PJRT as uint8/uint16 (axon transfer-safe)."""
    out = {}
    for k, v in derived.items():
        if v.dtype == F8:
            out[k] = v.view(np.uint8)
        elif v.dtype == BF:
            out[k] = v.view(np.uint16)
        else:
            out[k] = v
    return out


def kernel(**inputs):
    global DERIVED_SPECS
    derived = host_prep(inputs)
    if DERIVED_SPECS is None:
        DERIVED_SPECS = {k: v for k, v in derived.items()}
    nc = _get_nc()
    in_maps = make_in_maps(inputs, derived)
    res = run_bass_kernel_spmd(nc, in_maps, list(range(NCORES)))
    out = np.concatenate([res.results[c]["out"] for c in range(NCORES)], axis=0)
    return out.astype(np.float32)


# revision 4
# speedup vs baseline: 1.1757x; 1.1757x over previous
"""Trainium2 Bass kernel for nn_AttentionNet — v2.

Sharding: data-parallel over batch, 4 batches/core x 8 cores.

Encoder design (per batch):
  - src loaded NATURAL [tok, E] (fast DMA), LN1 computed along free axis on
    DVE (g/b folded into QKV weights host-side), then PE-transposed.
  - Scores in [keys, q] layout via fp8 DoubleRow matmuls; the -30*mask add
    rides the same PSUM accumulation as DoubleRow pairs (lhsT = mask slice,
    rhs = const [-30I|0 ; 0|-30I]) so masking costs no extra engine pass.
  - exp on ACT -> fp8 SBUF (eu8); AV + replicated-denominator matmuls in
    DoubleRow; softmax normalize = DVE reciprocal + multiply.
  - FFN w2 in DoubleRow fp8; LN2/dec-LN gains folded into w1 host-side.
Decoder + pointer: bf16, ACT Tanh; masked -10000 path kept exact in f32.
"""

import math

import numpy as np
import ml_dtypes

import concourse.bacc as bacc
import concourse.bass as bass
import concourse.tile as tile
from concourse import mybir
from concourse.bass_utils import run_bass_kernel_spmd

F32 = mybir.dt.float32
BF16 = mybir.dt.bfloat16
FP8 = mybir.dt.float8e4
I32 = mybir.dt.int32
AF = mybir.ActivationFunctionType
OP = mybir.AluOpType
PM = mybir.MatmulPerfMode

E, H, D, FF = 128, 4, 32, 512
HD = H * D
B, T, Q = 32, 1024, 1
NCORES = 8
BPC = B // NCORES
NKT = T // 128   # key tiles
NKP = NKT // 2   # key-tile pairs
SC = 1.0 / math.sqrt(D)
NMSK = -30.0     # mask additive constant (exp(-30*SC... no: exp(SC*(-30)) happens
                 # AFTER scale; we bake scale so exp sees SC*S - 30*SC*m... see below)

BF = ml_dtypes.bfloat16
F8 = ml_dtypes.float8_e4m3


def rep2(ap_):
    """Repeat a [P, F] AP twice along a new middle free dim via stride 0."""
    return bass.AP(tensor=ap_.tensor, offset=ap_.offset,
                   ap=[ap_.ap[0], [0, 2]] + list(ap_.ap[1:]))


def host_prep(inputs):
    """Fold LN gains/biases into weights; pre-cast/lay out weights."""
    inp = {k: np.asarray(v) for k, v in inputs.items()}
    d = {}
    for pfx in ("enc", "dec"):
        g1 = inp[f"{pfx}_ln1_g"].astype(np.float64)
        b1 = inp[f"{pfx}_ln1_b"].astype(np.float64)
        g2 = inp[f"{pfx}_ln2_g"].astype(np.float64)
        b2 = inp[f"{pfx}_ln2_b"].astype(np.float64)
        assert np.allclose(b1, 0) and np.allclose(b2, 0), "ln bias fold not implemented"
        for nm in ("wq", "wk", "wv"):
            w = inp[f"{pfx}_{nm}"].astype(np.float64)  # [H, E, D]
            w = w * g1[None, :, None]
            # -> [E, H*D]
            d[f"{pfx}_{nm}8"] = np.ascontiguousarray(
                w.transpose(1, 0, 2).reshape(E, HD)).astype(F8)
            d[f"{pfx}_{nm}16"] = np.ascontiguousarray(
                w.transpose(1, 0, 2).reshape(E, HD)).astype(BF)
        # wo: [H, D, E] -> [D, H, E] (lhsT tiles [32, h, E] at base 0)
        wo = inp[f"{pfx}_wo"].astype(np.float64).transpose(1, 0, 2)
        d[f"{pfx}_wo16"] = np.ascontiguousarray(wo).astype(BF)
        d[f"{pfx}_wo16f"] = np.ascontiguousarray(
            inp[f"{pfx}_wo"].astype(np.float64).reshape(HD, E)).astype(BF)
        # ffn w1 [E, FF] with g2 fold
        w1 = inp[f"{pfx}_ffn_w1"].astype(np.float64) * g2[:, None]
        d[f"{pfx}_w116"] = np.ascontiguousarray(w1).astype(BF)
        d[f"{pfx}_b1c"] = np.ascontiguousarray(
            inp[f"{pfx}_ffn_b1"].reshape(4, 128).T).astype(np.float32)  # [128, 4]
        # w2 [FF, E] -> DR-paired [128, pair 2, i 2, E]
        w2 = inp[f"{pfx}_ffn_w2"].astype(np.float64).reshape(2, 2, 128, E)
        d[f"{pfx}_w28"] = np.ascontiguousarray(w2.transpose(2, 0, 1, 3)).astype(F8)
        d[f"{pfx}_w216"] = np.ascontiguousarray(w2.transpose(2, 0, 1, 3)).astype(BF)
        d[f"{pfx}_b2c"] = np.ascontiguousarray(
            inp[f"{pfx}_ffn_b2"].reshape(E, 1)).astype(np.float32)
    d["ptr_wq16"] = inp["ptr_wq"].astype(BF)
    d["ptr_wk16"] = inp["ptr_wk"].astype(BF)

    # constants
    ipair = np.zeros((128, 2, 256), np.float32)
    for i in range(2):
        for r in range(128):
            ipair[r, i, 128 * i + r] = NMSK
    d["c_ipair"] = ipair.astype(F8)
    d["c_ident16"] = np.eye(128).astype(BF)
    d["c_ones8"] = np.ones((128, 2, 32), np.float32).astype(F8)
    d["c_onesrow"] = np.ones((1, 128), np.float32).astype(BF)
    # ind8 for partition-axis LN stats (batch b sums -> row b, sumsq -> 32+b)
    a = np.zeros((128, 8 * 36), np.float32)
    for b in range(4):
        a[:, 36 * b + b] = 1.0
        a[:, 36 * (4 + b) + 32 + b] = 1.0
    d["c_ind8"] = a.astype(BF)
    # ind2 for small partition-axis LN (sum -> row 0, sumsq -> row 32)
    a = np.zeros((128, 66), np.float32)
    a[:, 0] = 1.0
    a[:, 33 + 32] = 1.0
    d["c_ind2"] = a.astype(BF)
    d["c_eps4"] = np.full((4, 1), 1e-5, np.float32)
    d["c_eps128"] = np.full((128, 1), 1e-5, np.float32)
    d["c_negC"] = np.full((128, 1), -4.0, np.float32)
    d["c_eps1"] = np.full((1, 1), 1e-5, np.float32)
    # dsel[k, 4b+h] = 1 iff k == b  (decoder mask select)
    a = np.zeros((4, 16), np.float32)
    for b in range(BPC):
        a[b, 4 * b:4 * (b + 1)] = 1.0
    d["c_dsel"] = a.astype(BF)
    d["c_ones_col"] = np.ones((128, 1), np.float32).astype(BF)
    a = np.zeros((4, 4, 128), np.float32)
    for b in range(BPC):
        a[b, b, :] = 1.0
    d["c_rowsel"] = a.astype(BF)
    a = np.zeros((32, 4), np.float32)
    for i in range(32):
        a[i, i % 4] = 1.0
    d["c_p32"] = a.astype(np.float32)
    a = np.zeros((4, 128), np.float32)
    for h in range(H):
        a[h, 32 * h:32 * (h + 1)] = 1.0
    d["c_e4t"] = a.astype(np.float32)
    return d


DERIVED_SPECS = None  # filled on first host_prep


def _emit(nc, tc, tens, ctx):
    import os
    _dbg = bool(os.environ.get("KDBG"))
    _kph = int(os.environ.get("KPH", "9"))
    singles = ctx.enter_context(tc.tile_pool(name="singles", bufs=1))
    psum = ctx.enter_context(tc.tile_pool(name="psum", bufs=1, space="PSUM"))
    big = ctx.enter_context(tc.tile_pool(name="big", bufs=1))
    scr = ctx.enter_context(tc.tile_pool(name="scr", bufs=1))

    cnt = [0]

    def ps_S():  # [128, 1024] f32: scores / V-proj / srcT staging (2 banks)
        cnt[0] += 1
        return psum.tile([128, 1024], F32, tag="S", name=f"S{cnt[0]}", bufs=2)

    def ps_T():  # [128, 1024] f32 transposes (shares S tag)
        return ps_S()

    def ps_a():  # [32, 512] f32 AV (1 bank)
        cnt[0] += 1
        return psum.tile([32, 512], F32, tag="a", name=f"a{cnt[0]}", bufs=2)

    def ps_d():  # [32, 512] f32 denom (1 bank)
        cnt[0] += 1
        return psum.tile([32, 512], F32, tag="d", name=f"d{cnt[0]}", bufs=1)

    def ps_G():  # [128, 512] f32 general (1 bank)
        cnt[0] += 1
        return psum.tile([128, 512], F32, tag="G", name=f"G{cnt[0]}", bufs=1)

    def load(name, shape, dt, ap=None):
        tl = singles.tile(shape, dt, tag=name, name=name)
        src_ap = tens[name].ap() if ap is None else ap
        if dt in (FP8, BF16) and src_ap.dtype != dt:
            src_ap = src_ap.bitcast(dt)
        nc.sync.dma_start(out=tl[:], in_=src_ap)
        return tl

    # ---- weights / constants to SBUF ----
    w = {}
    for pfx in ("enc", "dec"):
        for nm in ("wq8", "wk8", "wv8"):
            w[f"{pfx}_{nm}"] = load(f"{pfx}_{nm}", [E, HD], FP8)
        for nm in ("wq16", "wk16", "wv16"):
            w[f"{pfx}_{nm}"] = load(f"{pfx}_{nm}", [E, HD], BF16)
        w[f"{pfx}_wo16"] = load(f"{pfx}_wo16", [D, H, E], BF16)
        w[f"{pfx}_wo16f"] = load(f"{pfx}_wo16f", [HD, E], BF16)
        w[f"{pfx}_w116"] = load(f"{pfx}_w116", [E, FF], BF16)
        w[f"{pfx}_b1c"] = load(f"{pfx}_b1c", [128, 4], F32)
        w[f"{pfx}_w28"] = load(f"{pfx}_w28", [128, 2, 2, E], FP8)
        w[f"{pfx}_w216"] = load(f"{pfx}_w216", [128, 2, 2, E], BF16)
        w[f"{pfx}_b2c"] = load(f"{pfx}_b2c", [E, 1], F32)
    ptrq = load("ptr_wq16", [E, E], BF16)
    ptrk = load("ptr_wk16", [E, E], BF16)
    ipair = load("c_ipair", [128, 2, 256], FP8)
    ident = load("c_ident16", [128, 128], BF16)
    ones8 = load("c_ones8", [128, 2, 32], FP8)
    onesrow = load("c_onesrow", [1, 128], BF16)
    ind8 = load("c_ind8", [128, 288], BF16)
    ind2 = load("c_ind2", [128, 66], BF16)
    eps4 = load("c_eps4", [4, 1], F32)
    eps128 = load("c_eps128", [128, 1], F32)
    negC = load("c_negC", [128, 1], F32)
    eps1 = load("c_eps1", [1, 1], F32)
    dsel = load("c_dsel", [4, 16], BF16)
    ones_col = load("c_ones_col", [128, 1], BF16)
    p32 = load("c_p32", [32, 4], F32)
    rowsel = load("c_rowsel", [4, 4, 128], BF16)
    e4t = load("c_e4t", [4, 128], F32)

    # persistent per-batch outputs
    srcTb = [big.tile([E, T], BF16, tag=f"srcT{b}", name=f"srcT{b}") for b in range(BPC)]
    h1b = [big.tile([E, T], BF16, tag=f"h1{b}", name=f"h1{b}") for b in range(BPC)]
    memb = [big.tile([E, T], BF16, tag=f"mem{b}", name=f"memb{b}") for b in range(BPC)]

    # ---------------- encoder: per batch ----------------
    for b in range(BPC):
        with nc.named_scope(f"enc_b{b}"):
            # src natural [128, 8, 128] f32 (one DMA)
            xsrc = scr.tile([128, NKT, E], F32, tag="xsrc", bufs=2)
            nc.sync.dma_start(out=xsrc[:],
                              in_=tens["src"].ap()[b].rearrange("(c p) e -> p c e", p=128))
            # masks: 2 DMAs of [128, 4, 1024] i32, cast to fp8 on Pool
            m8 = scr.tile([128, NKT, T], FP8, tag="m8", bufs=1 if _dbg else 2)
            for quar in range(4):
                mi = scr.tile([128, 2, T], I32, tag="mi", bufs=2)
                nc.sync.dma_start(
                    out=mi[:],
                    in_=tens["enc_mask"].ap()[b].rearrange(
                        "(c p) t -> p c t", p=128)[:, 2 * quar:2 * (quar + 1), :])
                nc.gpsimd.tensor_copy(m8[:, 2 * quar:2 * (quar + 1), :], mi[:])

            # LN1 along free axis (per 128-token tile)
            ssum = scr.tile([128, NKT], F32, tag="ssum")
            ssq = scr.tile([128, NKT], F32, tag="ssq")
            sqscr = scr.tile([128, NKT, E], F32, tag="sqscr")
            nc.vector.tensor_reduce(ssum[:].rearrange("p (t o) -> p t o", o=1),
                                    xsrc[:], axis=mybir.AxisListType.X, op=OP.add)
            nc.vector.tensor_mul(sqscr[:], xsrc[:], xsrc[:])
            nc.vector.tensor_reduce(ssq[:].rearrange("p (t o) -> p t o", o=1),
                                    sqscr[:], axis=mybir.AxisListType.X, op=OP.add)
            mcol = scr.tile([128, NKT], F32, tag="mcol")
            nc.vector.tensor_scalar_mul(mcol[:], ssum[:], 1.0 / E)
            var = scr.tile([128, NKT], F32, tag="var")
            nc.vector.scalar_tensor_tensor(out=var[:], in0=ssq[:], scalar=1.0 / E,
                                           in1=mcol[:], op0=OP.mult, op1=OP.bypass)
            msq = scr.tile([128, NKT], F32, tag="msq")
            nc.vector.tensor_mul(msq[:], mcol[:], mcol[:])
            nc.vector.tensor_sub(var[:], var[:], msq[:])
            sd = scr.tile([128, NKT], F32, tag="sd")
            nc.scalar.activation(out=sd[:], in_=var[:], func=AF.Sqrt, bias=eps128[:, 0:1])
            rs = scr.tile([128, NKT], F32, tag="rs")
            nc.vector.reciprocal(rs[:], sd[:])
            # xln bf16 tiles + PE transpose into XLT (bf16 psum), srcT (bf16)
            xln = scr.tile([128, NKT, E], BF16, tag="xln")
            for t in range(NKT):
                nc.vector.tensor_scalar(out=xln[:, t, :], in0=xsrc[:, t, :],
                                        scalar1=mcol[:, t:t + 1], scalar2=rs[:, t:t + 1],
                                        op0=OP.subtract, op1=OP.mult)
            xlt_ps = ps_T().bitcast(BF16)
            for t in range(NKT):
                nc.tensor.matmul(xlt_ps[:, 128 * t:128 * (t + 1)], xln[:, t, :],
                                 ident[:], start=True, stop=True, is_transpose=True)
            xlt8 = scr.tile([E, T], FP8, tag="xlt8")
            nc.vector.tensor_copy(xlt8[:], xlt_ps[:, 0:T])
            # src transpose for residual (bf16)
            xsb = scr.tile([128, NKT, E], BF16, tag="xsb")
            nc.vector.tensor_copy(xsb[:], xsrc[:])
            st_ps = ps_T().bitcast(BF16)
            for t in range(NKT):
                nc.tensor.matmul(st_ps[:, 128 * t:128 * (t + 1)], xsb[:, t, :],
                                 ident[:], start=True, stop=True, is_transpose=True)
            nc.vector.tensor_copy(srcTb[b][:], st_ps[:, 0:T])

            # ---- QKV (head-pair split so lhsT/rhs bases land on 0/32) ----
            # q8z: [64, hp 2, qp 4, i 2, 256] zero-interleaved, SC prefolded
            q8z = scr.tile([64, 2, 4, 2, 256], FP8, tag="q8z")
            nc.vector.memset(q8z[:], 0.0)
            # k8h: [64, hp 2, T]
            k8h = scr.tile([64, 2, T], FP8, tag="k8h")
            for c in range(2):
                s = slice(512 * c, 512 * (c + 1))
                for hp in range(2):
                    hs = slice(64 * hp, 64 * (hp + 1))
                    qp = ps_G()
                    nc.tensor.matmul(qp[0:64, :], w["enc_wq8"][:, hs], xlt8[:, s],
                                     start=True, stop=True)
                    base = q8z[:].rearrange("p a b i f -> p (a b i f)")
                    dst = bass.AP(tensor=base.tensor,
                                  offset=base.offset + 2048 * hp + 1024 * c,
                                  ap=[base.ap[0], [512, 2], [384, 2], [1, 128]])
                    nc.vector.tensor_scalar_mul(
                        dst, qp[0:64, :].rearrange("p (a i f) -> p a i f", a=2, i=2), SC)
                    kp_ = ps_G()
                    nc.tensor.matmul(kp_[0:64, :], w["enc_wk8"][:, hs], xlt8[:, s],
                                     start=True, stop=True)
                    nc.vector.tensor_copy(k8h[:, hp, s], kp_[0:64, :])
            # V natural (+ ones cols prefilled)
            v8 = scr.tile([128, NKP, 2, H, 32], FP8, tag="v8")
            vp = ps_S()
            for t in range(NKT):
                nc.tensor.matmul(vp[:, 128 * t:128 * (t + 1)],
                                 xlt8[:, 128 * t:128 * (t + 1)], w["enc_wv8"][:],
                                 start=True, stop=True)
            nc.vector.tensor_copy(
                v8[:].rearrange("p kp i h d -> p (kp i h d)"), vp[:])

            if _kph < 2:
                continue
            # ---- scores + mask + exp ----
            eu8 = scr.tile([128, NKP, 2, H, T], FP8, tag="eu8", bufs=1)
            for h in range(H):
                hb = slice(32 * (h % 2), 32 * (h % 2) + 32)
                krow = k8h[:, h // 2, :]
                for kt in range(NKT):
                    Sp = ps_S()
                    for qp_i in range(4):
                        qs = slice(256 * qp_i, 256 * (qp_i + 1))
                        nc.tensor.matmul(Sp[:, qs],
                                         rep2(krow[hb, 128 * kt:128 * (kt + 1)]),
                                         q8z[hb, h // 2, qp_i, :, :],
                                         start=True, stop=False, perf_mode=PM.DoubleRow)
                        nc.tensor.matmul(Sp[:, qs],
                                         m8[:, 2 * qp_i:2 * qp_i + 2,
                                            128 * kt:128 * (kt + 1)],
                                         ipair[:], start=False, stop=True,
                                         perf_mode=PM.DoubleRow)
                    nc.scalar.activation(out=eu8[:, kt // 2, kt % 2, h, :], in_=Sp[:],
                                         func=AF.Exp, bias=negC[:, 0:1])
                    if b == 0 and h == 0 and kt == 0 and "dbg_S" in tens:
                        Ssb = scr.tile([128, T], F32, tag="dbg")
                        nc.vector.tensor_copy(Ssb[:], Sp[:])
                        nc.sync.dma_start(out=tens["dbg_S"].ap(), in_=Ssb[:])
            # NOTE: mask adds NMSK (not NMSK*SC) because q8z is pre-scaled by SC
            # and the exp has scale=1 -> exp(SC*K.Q + NMSK*m). exp(-30) ~ 1e-13.

            if _kph < 3:
                continue
            # ---- AV + denom + normalize + wo ----
            for c in range(2):
                s = slice(512 * c, 512 * (c + 1))
                h1p = ps_G()
                for h in range(H):
                    av = ps_a()
                    dn = ps_d()
                    for kp_i in range(NKP):
                        nc.tensor.matmul(av[:], v8[:, kp_i, :, h, :],
                                         eu8[:, kp_i, :, h, s],
                                         start=(kp_i == 0), stop=(kp_i == NKP - 1),
                                         perf_mode=PM.DoubleRow)
                        nc.tensor.matmul(dn[:], ones8[:],
                                         eu8[:, kp_i, :, h, s],
                                         start=(kp_i == 0), stop=(kp_i == NKP - 1),
                                         perf_mode=PM.DoubleRow)
                    rc = scr.tile([32, 512], F32, tag="rc")
                    nc.vector.reciprocal(rc[:], dn[:])
                    hn = scr.tile([32, 512], BF16, tag="hn")
                    nc.vector.tensor_tensor(out=hn[:], in0=av[:], in1=rc[:], op=OP.mult)
                    if b == 0 and h == 0 and c == 0 and "dbg_dn" in tens:
                        dsb = scr.tile([32, 512], F32, tag="dbg2")
                        nc.vector.tensor_copy(dsb[:], dn[:])
                        nc.sync.dma_start(out=tens["dbg_dn"].ap(), in_=dsb[:])
                        asb = scr.tile([32, 512], F32, tag="dbg2")
                        nc.vector.tensor_copy(asb[:], av[:])
                        nc.sync.dma_start(out=tens["dbg_av"].ap(), in_=asb[:])
                    nc.tensor.matmul(h1p[:], w["enc_wo16"][:, h, :], hn[:],
                                     start=(h == 0), stop=(h == H - 1))
                nc.vector.tensor_add(h1b[b][:, s], h1p[:], srcTb[b][:, s])
            if b == 0 and "dbg_h1" in tens:
                h1f = scr.tile([128, T], F32, tag="dbg")
                nc.vector.tensor_copy(h1f[:], h1b[0][:])
                nc.sync.dma_start(out=tens["dbg_h1"].ap(), in_=h1f[:])
                x8f = scr.tile([128, T], F32, tag="dbg")
                nc.vector.tensor_copy(x8f[:], xlt8[:])
                nc.sync.dma_start(out=tens["dbg_xlt"].ap(), in_=x8f[:])

    if _kph < 5:
        stub = scr.tile([BPC, T], F32, tag="lg_m", name="stub", bufs=1)
        hsrc = h1b[0] if _kph >= 3 else srcTb[0]
        nc.vector.tensor_copy(stub[:], hsrc[0:BPC, :])
        nc.sync.dma_start(out=tens["out"].ap().rearrange("b q t -> (b q) t"), in_=stub[:])
        return

    # ---------------- encoder LN2 + FFN (all batches) ----------------
    def ln_stats_group(xs, tagn):
        """Partition-axis LN core stats for 4 bf16 [E, T] tiles.
        Returns (mrow, nmrow, rsrow): [4, T] f32/bf16 tiles (per-batch rows)."""
        stats = ps_S()
        for b, xt in enumerate(xs):
            sq = scr.tile([128, T], BF16, tag="lg_sq", name=f"sq{tagn}", bufs=2)
            nc.vector.tensor_mul(sq[:], xt[:], xt[:])
            for c in range(2):
                s = slice(512 * c, 512 * (c + 1))
                nc.tensor.matmul(stats[0:36, s], ind8[:, 36 * b:36 * (b + 1)],
                                 xt[:, s], start=(b == 0), stop=False)
                nc.tensor.matmul(stats[0:36, s], ind8[:, 36 * (4 + b):36 * (5 + b)],
                                 sq[:, s], start=False,
                                 stop=(b == len(xs) - 1))
        m = scr.tile([4, T], F32, tag="lg_m", name=f"m{tagn}")
        nc.vector.tensor_scalar_mul(m[:], stats[0:4, :], 1.0 / E)
        var = scr.tile([4, T], F32, tag="lg_v", name=f"v{tagn}")
        nc.vector.tensor_scalar_mul(var[:], stats[32:36, :], 1.0 / E)
        msq = scr.tile([4, T], F32, tag="lg_ms", name=f"ms{tagn}")
        nc.vector.tensor_mul(msq[:], m[:], m[:])
        nc.vector.tensor_sub(var[:], var[:], msq[:])
        sd = scr.tile([4, T], F32, tag="lg_ms", name=f"sd{tagn}")
        nc.scalar.activation(out=sd[:], in_=var[:], func=AF.Sqrt, bias=eps4[:, 0:1])
        rsf = scr.tile([4, T], F32, tag="lg_v", name=f"rsf{tagn}")
        nc.vector.reciprocal(rsf[:], sd[:])
        rs = scr.tile([4, T], BF16, tag="lg_rs", name=f"rs{tagn}", bufs=2)
        nc.vector.tensor_copy(rs[:], rsf[:])
        nm = scr.tile([4, T], BF16, tag="lg_nm", name=f"nm{tagn}", bufs=2)
        nc.vector.scalar_tensor_tensor(out=nm[:], in0=m[:], scalar=-1.0,
                                       in1=rsf[:], op0=OP.mult, op1=OP.mult)
        return rs, nm

    def ln_apply(xt, rs, nm, b, c, out, out_slice=None):
        """out[:, s] = x*(ones x rs_b) + (ones x nm_b) for 512-chunk c."""
        s = slice(512 * c, 512 * (c + 1))
        a_ps = ps_G()
        nc.tensor.matmul(a_ps[:], rowsel[:, b, :], rs[0:4, s], start=True, stop=True)
        b_ps = ps_G()
        nc.tensor.matmul(b_ps[:], rowsel[:, b, :], nm[0:4, s], start=True, stop=True)
        tmp = scr.tile([128, 512], F32, tag="lntmp")
        nc.vector.scalar_tensor_tensor(out=tmp[:], in0=xt[:, s], scalar=1.0,
                                       in1=a_ps[:], op0=OP.bypass, op1=OP.mult)
        dst = out[:, s] if out_slice is None else out_slice
        nc.vector.scalar_tensor_tensor(out=dst, in0=tmp[:], scalar=1.0,
                                       in1=b_ps[:], op0=OP.bypass, op1=OP.add)

    with nc.named_scope("enc_ffn"):
        rs2, nm2 = ln_stats_group(h1b, "l2")
        for b in range(BPC):
            act8 = scr.tile([128, 2, 2, T], FP8, tag="act8")
            hln = scr.tile([E, T], BF16, tag="hln")
            for c in range(2):
                ln_apply(h1b[b], rs2, nm2, b, c, hln)
            for fc in range(4):
                for c in range(2):
                    s = slice(512 * c, 512 * (c + 1))
                    g = ps_G()
                    nc.tensor.matmul(g[:], w["enc_w116"][:, 128 * fc:128 * (fc + 1)],
                                     hln[:, s], start=True, stop=True)
                    nc.scalar.activation(out=act8[:, fc // 2, fc % 2, s], in_=g[:],
                                         func=AF.Relu, bias=w["enc_b1c"][:, fc:fc + 1])
            for c in range(2):
                s = slice(512 * c, 512 * (c + 1))
                mp = ps_G()
                for p in range(2):
                    nc.tensor.matmul(mp[:], w["enc_w28"][:, p, :, :],
                                     act8[:, p, :, s], start=(p == 0), stop=(p == 1),
                                     perf_mode=PM.DoubleRow)
                tmp2 = scr.tile([128, 512], F32, tag="ffntmp")
                nc.vector.tensor_scalar(out=tmp2[:], in0=mp[:],
                                        scalar1=w["enc_b2c"][:, 0:1], scalar2=None,
                                        op0=OP.add)
                nc.vector.tensor_add(memb[b][:, s], tmp2[:], h1b[b][:, s])

    if _kph < 6:
        stub = scr.tile([BPC, T], F32, tag="lg_m", name="stub", bufs=1)
        nc.vector.tensor_copy(stub[:], memb[0][0:BPC, :])
        nc.sync.dma_start(out=tens["out"].ap().rearrange("b q t -> (b q) t"), in_=stub[:])
        return

    # ---------------- decoder ----------------
    with nc.named_scope("decoder"):
        tgtT = singles.tile([E, BPC], F32, tag="tgtT")
        nc.sync.dma_start(out=tgtT[:], in_=tens["tgt"].ap().rearrange("b q e -> e (b q)"))
        dmi = scr.tile([BPC, T], I32, tag="dmi", bufs=1)
        nc.sync.dma_start(out=dmi[:], in_=tens["dec_mask"].ap().rearrange("b q t -> (b q) t"))
        dmf = singles.tile([BPC, T], BF16, tag="dmf")
        nc.vector.tensor_scalar_mul(dmf[:], dmi[:], -10000.0)
        dmf32 = singles.tile([BPC, T], F32, tag="dmf32")
        nc.vector.tensor_scalar_mul(dmf32[:], dmi[:], -10000.0)
        dwf = singles.tile([BPC, T], F32, tag="dwf")
        nc.vector.tensor_scalar(out=dwf[:], in0=dmi[:], scalar1=-1.0, scalar2=1.0,
                                op0=OP.mult, op1=OP.add)

        def ln_small(x, n, tagn):
            """Core LN over partitions for [E, n] f32 -> bf16 (g/b folded away)."""
            xbf = scr.tile([E, BPC], BF16, tag=f"dx{tagn}")
            nc.vector.tensor_copy(xbf[:, :n], x[:, :n])
            sq = scr.tile([E, BPC], BF16, tag=f"dq{tagn}")
            nc.vector.tensor_mul(sq[:, :n], xbf[:, :n], xbf[:, :n])
            stp = ps_G()
            nc.tensor.matmul(stp[0:33, :n], ind2[:, 0:33], xbf[:, :n], start=True, stop=False)
            nc.tensor.matmul(stp[0:33, :n], ind2[:, 33:66], sq[:, :n], start=False, stop=True)
            st = scr.tile([33, BPC], F32, tag=f"ds{tagn}")
            nc.vector.tensor_copy(st[:, :n], stp[0:33, :n])
            mn = scr.tile([1, BPC], F32, tag=f"dm{tagn}")
            nc.vector.tensor_scalar_mul(mn[:, :n], st[0:1, :n], 1.0 / E)
            msq = scr.tile([1, BPC], F32, tag=f"dmq{tagn}")
            nc.vector.tensor_mul(msq[:, :n], mn[:, :n], mn[:, :n])
            var = scr.tile([1, BPC], F32, tag=f"dv{tagn}")
            nc.vector.tensor_scalar_mul(var[:, :n], st[32:33, :n], 1.0 / E)
            nc.vector.tensor_sub(var[:, :n], var[:, :n], msq[:, :n])
            sdd = scr.tile([1, BPC], F32, tag=f"dsd{tagn}")
            nc.scalar.activation(out=sdd[:, :n], in_=var[:, :n], func=AF.Sqrt,
                                 bias=eps1[:, 0:1])
            rsf = scr.tile([1, BPC], F32, tag=f"drf{tagn}")
            nc.vector.reciprocal(rsf[:, :n], sdd[:, :n])
            rs = scr.tile([1, BPC], BF16, tag=f"dr{tagn}")
            nc.vector.tensor_copy(rs[:, :n], rsf[:, :n])
            nm = scr.tile([1, BPC], BF16, tag=f"dn{tagn}")
            nc.vector.scalar_tensor_tensor(out=nm[:, :n], in0=mn[:, :n], scalar=-1.0,
                                           in1=rsf[:, :n], op0=OP.mult, op1=OP.mult)
            a_ps = ps_G()
            nc.tensor.matmul(a_ps[:, 0:n], onesrow[:], rs[:, :n], start=True, stop=True)
            b_ps = ps_G()
            nc.tensor.matmul(b_ps[:, 0:n], onesrow[:], nm[:, :n], start=True, stop=True)
            tmp = scr.tile([E, BPC], F32, tag=f"dt{tagn}")
            nc.vector.scalar_tensor_tensor(out=tmp[:, :n], in0=x[:, :n], scalar=1.0,
                                           in1=a_ps[:, 0:n], op0=OP.bypass, op1=OP.mult)
            out = scr.tile([E, BPC], BF16, tag=f"do{tagn}")
            nc.vector.scalar_tensor_tensor(out=out[:, :n], in0=tmp[:, :n], scalar=1.0,
                                           in1=b_ps[:, 0:n], op0=OP.bypass, op1=OP.add)
            return out

        tln = ln_small(tgtT, BPC, "t")
        qd_ps = ps_G()
        nc.tensor.matmul(qd_ps[0:HD, 0:BPC], w["dec_wq16"][:], tln[:, :BPC],
                         start=True, stop=True)
        qdec = scr.tile([HD, BPC], BF16, tag="qdec")
        nc.vector.tensor_copy(qdec[:], qd_ps[0:HD, 0:BPC])

        rsm, nmm = ln_stats_group(memb, "lm")
        h1d = singles.tile([E, BPC], F32, tag="h1d")
        for b in range(BPC):
            mln = scr.tile([E, T], BF16, tag="mln")
            for c in range(2):
                ln_apply(memb[b], rsm, nmm, b, c, mln)
            kd = scr.tile([HD, T], BF16, tag="kd", bufs=1)
            for c in range(2):
                s = slice(512 * c, 512 * (c + 1))
                kp_ = ps_G()
                nc.tensor.matmul(kp_[:], w["dec_wk16"][:], mln[:, s], start=True, stop=True)
                nc.vector.tensor_copy(kd[:, s], kp_[:])
            vd = scr.tile([128, NKT, HD], BF16, tag="vd", bufs=1)
            vp2 = ps_S()
            for t in range(NKT):
                nc.tensor.matmul(vp2[:, 128 * t:128 * (t + 1)],
                                 mln[:, 128 * t:128 * (t + 1)], w["dec_wv16"][:],
                                 start=True, stop=True)
            nc.vector.tensor_copy(vd[:], vp2[:].rearrange("p (c e) -> p c e", c=NKT))
            qblk = scr.tile([HD, 4], BF16, tag="qblk")
            nc.vector.memset(qblk[:], 0.0)
            for h in range(H):
                nc.vector.tensor_copy(qblk[32 * h:32 * (h + 1), h:h + 1],
                                      qdec[32 * h:32 * (h + 1), b:b + 1])
            ud_ps = ps_G()
            for kt in range(NKT):
                cs = slice(4 * kt, 4 * (kt + 1))
                nc.tensor.matmul(ud_ps[:, cs], kd[:, 128 * kt:128 * (kt + 1)], qblk[:],
                                 start=True, stop=False)
                nc.tensor.matmul(ud_ps[:, cs], dmf[0:4, 128 * kt:128 * (kt + 1)],
                                 dsel[:, 4 * b:4 * (b + 1)], start=False, stop=True)
            eud = scr.tile([128, 4 * NKT], BF16, tag="eud")
            nc.scalar.activation(out=eud[:], in_=ud_ps[:, 0:4 * NKT], func=AF.Exp,
                                 scale=SC)
            d1_ps = ps_G()
            nc.tensor.matmul(d1_ps[0:32, 0:1], eud[:], ones_col[:], start=True, stop=True)
            d1 = scr.tile([32, 1], F32, tag="d1s")
            nc.vector.tensor_copy(d1[:], d1_ps[0:32, 0:1])
            d4_ps = ps_G()
            nc.tensor.matmul(d4_ps[0:4, 0:1], p32[:], d1[:], start=True, stop=True)
            rc4 = scr.tile([4, 1], F32, tag="rc4")
            nc.vector.reciprocal(rc4[:], d4_ps[0:4, 0:1])
            rb_ps = ps_G()
            nc.tensor.matmul(rb_ps[:, 0:1], e4t[:], rc4[:], start=True, stop=True)
            rb = scr.tile([128, 1], F32, tag="rb")
            nc.vector.tensor_copy(rb[:], rb_ps[:, 0:1])
            hd_ps = ps_G()
            for kt in range(NKT):
                nc.tensor.matmul(hd_ps[:, 0:4], vd[:, kt, :], eud[:, 4 * kt:4 * (kt + 1)],
                                 start=(kt == 0), stop=(kt == NKT - 1))
            hdec = scr.tile([HD, 1], BF16, tag="hdec")
            for h in range(H):
                nc.vector.tensor_copy(hdec[32 * h:32 * (h + 1), 0:1],
                                      hd_ps[32 * h:32 * (h + 1), h:h + 1])
            nc.vector.tensor_scalar_mul(hdec[:], hdec[:], rb[:, 0:1])
            ao_ps = ps_G()
            nc.tensor.matmul(ao_ps[:, 0:1], w["dec_wo16f"][:], hdec[:],
                             start=True, stop=True)
            nc.vector.tensor_add(h1d[:, b:b + 1], ao_ps[:, 0:1], tgtT[:, b:b + 1])

        hln2d = ln_small(h1d, BPC, "d2")
        dact_ps = ps_G()
        for fc in range(4):
            nc.tensor.matmul(dact_ps[:, 4 * fc:4 * (fc + 1)],
                             w["dec_w116"][:, 128 * fc:128 * (fc + 1)], hln2d[:, :BPC],
                             start=True, stop=True)
        dact = scr.tile([128, 2, 2, 4], FP8, tag="dact")
        for fc in range(4):
            nc.vector.tensor_scalar(out=dact[:, fc // 2, fc % 2, :],
                                    in0=dact_ps[:, 4 * fc:4 * (fc + 1)],
                                    scalar1=w["dec_b1c"][:, fc:fc + 1],
                                    scalar2=0.0, op0=OP.add, op1=OP.max)
        do_ps = ps_G()
        for p in range(2):
            nc.tensor.matmul(do_ps[:, 0:BPC], w["dec_w28"][:, p, :, :],
                             dact[:, p, :, :], start=(p == 0), stop=(p == 1),
                             perf_mode=PM.DoubleRow)
        decT = singles.tile([E, BPC], F32, tag="decT")
        tmp2 = scr.tile([E, BPC], F32, tag="dtmp2")
        nc.vector.tensor_scalar(out=tmp2[:], in0=do_ps[:, 0:BPC],
                                scalar1=w["dec_b2c"][:, 0:1], scalar2=None, op0=OP.add)
        nc.vector.tensor_add(decT[:], tmp2[:], h1d[:])

    if _kph < 7:
        stub = scr.tile([BPC, T], F32, tag="lg_m", name="stub", bufs=1)
        nc.vector.tensor_copy(stub[:], memb[0][0:BPC, :])
        nc.sync.dma_start(out=tens["out"].ap().rearrange("b q t -> (b q) t"), in_=stub[:])
        return

    # ---------------- pointer ----------------
    with nc.named_scope("pointer"):
        dec16 = scr.tile([E, BPC], BF16, tag="dec16")
        nc.vector.tensor_copy(dec16[:], decT[:])
        qp_ps = ps_G()
        nc.tensor.matmul(qp_ps[:, 0:BPC], ptrq[:], dec16[:], start=True, stop=True)
        qpi = scr.tile([E, 4 * BPC], BF16, tag="qpi")
        nc.vector.memset(qpi[:], 0.0)
        for b in range(BPC):
            nc.vector.tensor_copy(qpi[:, 5 * b:5 * b + 1], qp_ps[:, b:b + 1])
        up_ps = ps_S()
        for b in range(BPC):
            kp8 = scr.tile([E, T], BF16, tag="kp8", bufs=1)
            for c in range(2):
                s = slice(512 * c, 512 * (c + 1))
                kpc = ps_G()
                nc.tensor.matmul(kpc[:], ptrk[:], memb[b][:, s], start=True, stop=True)
                nc.vector.tensor_copy(kp8[:, s], kpc[:])
            for c in range(2):
                s = slice(512 * c, 512 * (c + 1))
                nc.tensor.matmul(up_ps[0:BPC, s], qpi[:, 4 * b:4 * (b + 1)], kp8[:, s],
                                 start=(b == 0), stop=(b == BPC - 1))
        # L = 10*tanh(U/sqrt(E)); masked -> *dwf + dmf32; log_softmax
        th = scr.tile([BPC, T], F32, tag="th", bufs=1)
        nc.scalar.activation(out=th[:], in_=up_ps[0:BPC, :], func=AF.Tanh,
                             scale=1.0 / math.sqrt(E))
        L = scr.tile([BPC, T], F32, tag="L", bufs=1)
        nc.vector.scalar_tensor_tensor(out=L[:], in0=th[:], scalar=10.0,
                                       in1=dwf[:], op0=OP.mult, op1=OP.mult)
        nc.vector.tensor_add(L[:], L[:], dmf32[:])
        et = scr.tile([BPC, T], F32, tag="lg_ms", name="et", bufs=1)
        se = scr.tile([BPC, 1], F32, tag="se")
        nc.scalar.activation(out=et[:], in_=L[:], func=AF.Exp, accum_out=se[:])
        lse = scr.tile([BPC, 1], F32, tag="lse")
        nc.scalar.activation(out=lse[:], in_=se[:], func=AF.Ln)
        res = scr.tile([BPC, T], F32, tag="lg_m", name="res", bufs=1)
        nc.vector.tensor_scalar(out=res[:], in0=L[:], scalar1=lse[:, 0:1], scalar2=None,
                                op0=OP.subtract)
        nc.sync.dma_start(out=tens["out"].ap().rearrange("b q t -> (b q) t"), in_=res[:])


def build():
    import contextlib
    global DERIVED_SPECS
    nc = bacc.Bacc()
    tens = {}
    tens["src"] = nc.dram_tensor("src", [BPC, T, E], F32, kind="ExternalInput")
    tens["tgt"] = nc.dram_tensor("tgt", [BPC, Q, E], F32, kind="ExternalInput")
    tens["enc_mask"] = nc.dram_tensor("enc_mask", [BPC, T, T], I32, kind="ExternalInput")
    tens["dec_mask"] = nc.dram_tensor("dec_mask", [BPC, Q, T], I32, kind="ExternalInput")
    for name, arr in DERIVED_SPECS.items():
        if arr.dtype == F8:
            dt_ = mybir.dt.uint8
        elif arr.dtype == BF:
            dt_ = mybir.dt.uint16
        else:
            dt_ = mybir.dt.from_np(arr.dtype)
        tens[name] = nc.dram_tensor(name, list(arr.shape), dt_, kind="ExternalInput")
    tens["out"] = nc.dram_tensor("out", [BPC, Q, T], F32, kind="ExternalOutput")
    import os
    if os.environ.get("KDBG"):
        for nm, shp in [("dbg_S", [128, T]), ("dbg_dn", [32, 512]),
                        ("dbg_av", [32, 512]), ("dbg_h1", [128, T]),
                        ("dbg_xlt", [128, T])]:
            tens[nm] = nc.dram_tensor(nm, shp, F32, kind="ExternalOutput")

    with tile.TileContext(nc) as tc:
        with contextlib.ExitStack() as ctx:
            _emit(nc, tc, tens, ctx)
    nc.finalize()
    return nc


_built = {}


def _get_nc():
    if "nc" not in _built:
        _built["nc"] = build()
    return _built["nc"]


def make_in_maps(inputs, derived):
    in_maps = []
    for c in range(NCORES):
        s = slice(BPC * c, BPC * (c + 1))
        m = {
            "src": np.ascontiguousarray(np.asarray(inputs["src"])[s]),
            "tgt": np.ascontiguousarray(np.asarray(inputs["tgt"])[s]),
            "enc_mask": np.ascontiguousarray(np.asarray(inputs["enc_mask"])[s]),
            "dec_mask": np.ascontiguousarray(np.asarray(inputs["dec_mask"])[s]),
        }
        m.update(_wire_dtypes(derived))
        in_maps.append(m)
    return in_maps


def _wire_dtypes(derived):
    """fp8/bf16 arrays cross PJRT as uint8/uint16 (axon transfer-safe)."""
    out = {}
    for k, v in derived.items():
        if v.dtype == F8:
            out[k] = v.view(np.uint8)
        elif v.dtype == BF:
            out[k] = v.view(np.uint16)
        else:
            out[k] = v
    return out


def kernel(**inputs):
    global DERIVED_SPECS
    derived = host_prep(inputs)
    if DERIVED_SPECS is None:
        DERIVED_SPECS = {k: v for k, v in derived.items()}
    nc = _get_nc()
    in_maps = make_in_maps(inputs, derived)
    res = run_bass_kernel_spmd(nc, in_maps, list(range(NCORES)))
    out = np.concatenate([res.results[c]["out"] for c in range(NCORES)], axis=0)
    return out.astype(np.float32)


# revision 5
# speedup vs baseline: 1.1775x; 1.0015x over previous
"""Trainium2 Bass kernel for nn_AttentionNet — v2.

Sharding: data-parallel over batch, 4 batches/core x 8 cores.

Encoder design (per batch):
  - src loaded NATURAL [tok, E] (fast DMA), LN1 computed along free axis on
    DVE (g/b folded into QKV weights host-side), then PE-transposed.
  - Scores in [keys, q] layout via fp8 DoubleRow matmuls; the -30*mask add
    rides the same PSUM accumulation as DoubleRow pairs (lhsT = mask slice,
    rhs = const [-30I|0 ; 0|-30I]) so masking costs no extra engine pass.
  - exp on ACT -> fp8 SBUF (eu8); AV + replicated-denominator matmuls in
    DoubleRow; softmax normalize = DVE reciprocal + multiply.
  - FFN w2 in DoubleRow fp8; LN2/dec-LN gains folded into w1 host-side.
Decoder + pointer: bf16, ACT Tanh; masked -10000 path kept exact in f32.
"""

import math

import numpy as np
import ml_dtypes

import concourse.bacc as bacc
import concourse.bass as bass
import concourse.tile as tile
from concourse import mybir
from concourse.bass_utils import run_bass_kernel_spmd

F32 = mybir.dt.float32
BF16 = mybir.dt.bfloat16
FP8 = mybir.dt.float8e4
I32 = mybir.dt.int32
AF = mybir.ActivationFunctionType
OP = mybir.AluOpType
PM = mybir.MatmulPerfMode

E, H, D, FF = 128, 4, 32, 512
HD = H * D
B, T, Q = 32, 1024, 1
NCORES = 8
BPC = B // NCORES
NKT = T // 128   # key tiles
NKP = NKT // 2   # key-tile pairs
SC = 1.0 / math.sqrt(D)
NMSK = -30.0     # mask additive constant (exp(-30*SC... no: exp(SC*(-30)) happens
                 # AFTER scale; we bake scale so exp sees SC*S - 30*SC*m... see below)

BF = ml_dtypes.bfloat16
F8 = ml_dtypes.float8_e4m3


def rep2(ap_):
    """Repeat a [P, F] AP twice along a new middle free dim via stride 0."""
    return bass.AP(tensor=ap_.tensor, offset=ap_.offset,
                   ap=[ap_.ap[0], [0, 2]] + list(ap_.ap[1:]))


def host_prep(inputs):
    """Fold LN gains/biases into weights; pre-cast/lay out weights."""
    inp = {k: np.asarray(v) for k, v in inputs.items()}
    d = {}
    for pfx in ("enc", "dec"):
        g1 = inp[f"{pfx}_ln1_g"].astype(np.float64)
        b1 = inp[f"{pfx}_ln1_b"].astype(np.float64)
        g2 = inp[f"{pfx}_ln2_g"].astype(np.float64)
        b2 = inp[f"{pfx}_ln2_b"].astype(np.float64)
        assert np.allclose(b1, 0) and np.allclose(b2, 0), "ln bias fold not implemented"
        for nm in ("wq", "wk", "wv"):
            w = inp[f"{pfx}_{nm}"].astype(np.float64)  # [H, E, D]
            w = w * g1[None, :, None]
            # -> [E, H*D]
            d[f"{pfx}_{nm}8"] = np.ascontiguousarray(
                w.transpose(1, 0, 2).reshape(E, HD)).astype(F8)
            d[f"{pfx}_{nm}16"] = np.ascontiguousarray(
                w.transpose(1, 0, 2).reshape(E, HD)).astype(BF)
        # wo: [H, D, E] -> [D, H, E] (lhsT tiles [32, h, E] at base 0)
        wo = inp[f"{pfx}_wo"].astype(np.float64).transpose(1, 0, 2)
        d[f"{pfx}_wo16"] = np.ascontiguousarray(wo).astype(BF)
        d[f"{pfx}_wo16f"] = np.ascontiguousarray(
            inp[f"{pfx}_wo"].astype(np.float64).reshape(HD, E)).astype(BF)
        # ffn w1 [E, FF] with g2 fold
        w1 = inp[f"{pfx}_ffn_w1"].astype(np.float64) * g2[:, None]
        d[f"{pfx}_w116"] = np.ascontiguousarray(w1).astype(BF)
        d[f"{pfx}_b1c"] = np.ascontiguousarray(
            inp[f"{pfx}_ffn_b1"].reshape(4, 128).T).astype(np.float32)  # [128, 4]
        # w2 [FF, E] -> DR-paired [128, pair 2, i 2, E]
        w2 = inp[f"{pfx}_ffn_w2"].astype(np.float64).reshape(2, 2, 128, E)
        d[f"{pfx}_w28"] = np.ascontiguousarray(w2.transpose(2, 0, 1, 3)).astype(F8)
        d[f"{pfx}_w216"] = np.ascontiguousarray(w2.transpose(2, 0, 1, 3)).astype(BF)
        d[f"{pfx}_b2c"] = np.ascontiguousarray(
            inp[f"{pfx}_ffn_b2"].reshape(E, 1)).astype(np.float32)
    d["ptr_wq16"] = inp["ptr_wq"].astype(BF)
    d["ptr_wk16"] = inp["ptr_wk"].astype(BF)

    # constants
    ipair = np.zeros((128, 2, 256), np.float32)
    for i in range(2):
        for r in range(128):
            ipair[r, i, 128 * i + r] = NMSK
    d["c_ipair"] = ipair.astype(F8)
    d["c_ident16"] = np.eye(128).astype(BF)
    d["c_ones8"] = np.ones((128, 2, 32), np.float32).astype(F8)
    d["c_onesrow"] = np.ones((1, 128), np.float32).astype(BF)
    # ind8 for partition-axis LN stats (batch b sums -> row b, sumsq -> 32+b)
    a = np.zeros((128, 8 * 36), np.float32)
    for b in range(4):
        a[:, 36 * b + b] = 1.0
        a[:, 36 * (4 + b) + 32 + b] = 1.0
    d["c_ind8"] = a.astype(BF)
    # ind2 for small partition-axis LN (sum -> row 0, sumsq -> row 32)
    a = np.zeros((128, 66), np.float32)
    a[:, 0] = 1.0
    a[:, 33 + 32] = 1.0
    d["c_ind2"] = a.astype(BF)
    d["c_eps4"] = np.full((4, 1), 1e-5, np.float32)
    d["c_eps128"] = np.full((128, 1), 1e-5, np.float32)
    d["c_negC"] = np.full((128, 1), -4.0, np.float32)
    d["c_eps1"] = np.full((1, 1), 1e-5, np.float32)
    # dsel[k, 4b+h] = 1 iff k == b  (decoder mask select)
    a = np.zeros((4, 16), np.float32)
    for b in range(BPC):
        a[b, 4 * b:4 * (b + 1)] = 1.0
    d["c_dsel"] = a.astype(BF)
    d["c_ones_col"] = np.ones((128, 1), np.float32).astype(BF)
    a = np.zeros((4, 4, 128), np.float32)
    for b in range(BPC):
        a[b, b, :] = 1.0
    d["c_rowsel"] = a.astype(BF)
    a = np.zeros((32, 4), np.float32)
    for i in range(32):
        a[i, i % 4] = 1.0
    d["c_p32"] = a.astype(np.float32)
    a = np.zeros((4, 128), np.float32)
    for h in range(H):
        a[h, 32 * h:32 * (h + 1)] = 1.0
    d["c_e4t"] = a.astype(np.float32)
    return d


DERIVED_SPECS = None  # filled on first host_prep


def _emit(nc, tc, tens, ctx):
    import os
    _dbg = bool(os.environ.get("KDBG"))
    _kph = int(os.environ.get("KPH", "9"))
    singles = ctx.enter_context(tc.tile_pool(name="singles", bufs=1))
    psum = ctx.enter_context(tc.tile_pool(name="psum", bufs=1, space="PSUM"))
    big = ctx.enter_context(tc.tile_pool(name="big", bufs=1))
    scr = ctx.enter_context(tc.tile_pool(name="scr", bufs=1))

    cnt = [0]

    def ps_S():  # [128, 1024] f32: scores / V-proj / srcT staging (2 banks)
        cnt[0] += 1
        return psum.tile([128, 1024], F32, tag="S", name=f"S{cnt[0]}", bufs=2)

    def ps_T():  # [128, 1024] f32 transposes (shares S tag)
        return ps_S()

    def ps_a():  # [32, 512] f32 AV (1 bank)
        cnt[0] += 1
        return psum.tile([32, 512], F32, tag="a", name=f"a{cnt[0]}", bufs=1)

    def ps_d():  # [32, 512] f32 denom (1 bank)
        cnt[0] += 1
        return psum.tile([32, 512], F32, tag="d", name=f"d{cnt[0]}", bufs=2)

    def ps_G():  # [128, 512] f32 general (1 bank)
        cnt[0] += 1
        return psum.tile([128, 512], F32, tag="G", name=f"G{cnt[0]}", bufs=1)

    def load(name, shape, dt, ap=None):
        tl = singles.tile(shape, dt, tag=name, name=name)
        src_ap = tens[name].ap() if ap is None else ap
        if dt in (FP8, BF16) and src_ap.dtype != dt:
            src_ap = src_ap.bitcast(dt)
        nc.sync.dma_start(out=tl[:], in_=src_ap)
        return tl

    # ---- weights / constants to SBUF ----
    w = {}
    for pfx in ("enc", "dec"):
        for nm in ("wq8", "wk8", "wv8"):
            w[f"{pfx}_{nm}"] = load(f"{pfx}_{nm}", [E, HD], FP8)
        for nm in ("wq16", "wk16", "wv16"):
            w[f"{pfx}_{nm}"] = load(f"{pfx}_{nm}", [E, HD], BF16)
        w[f"{pfx}_wo16"] = load(f"{pfx}_wo16", [D, H, E], BF16)
        w[f"{pfx}_wo16f"] = load(f"{pfx}_wo16f", [HD, E], BF16)
        w[f"{pfx}_w116"] = load(f"{pfx}_w116", [E, FF], BF16)
        w[f"{pfx}_b1c"] = load(f"{pfx}_b1c", [128, 4], F32)
        w[f"{pfx}_w28"] = load(f"{pfx}_w28", [128, 2, 2, E], FP8)
        w[f"{pfx}_w216"] = load(f"{pfx}_w216", [128, 2, 2, E], BF16)
        w[f"{pfx}_b2c"] = load(f"{pfx}_b2c", [E, 1], F32)
    ptrq = load("ptr_wq16", [E, E], BF16)
    ptrk = load("ptr_wk16", [E, E], BF16)
    ipair = load("c_ipair", [128, 2, 256], FP8)
    ident = load("c_ident16", [128, 128], BF16)
    ones8 = load("c_ones8", [128, 2, 32], FP8)
    onesrow = load("c_onesrow", [1, 128], BF16)
    ind8 = load("c_ind8", [128, 288], BF16)
    ind2 = load("c_ind2", [128, 66], BF16)
    eps4 = load("c_eps4", [4, 1], F32)
    eps128 = load("c_eps128", [128, 1], F32)
    negC = load("c_negC", [128, 1], F32)
    eps1 = load("c_eps1", [1, 1], F32)
    dsel = load("c_dsel", [4, 16], BF16)
    ones_col = load("c_ones_col", [128, 1], BF16)
    p32 = load("c_p32", [32, 4], F32)
    rowsel = load("c_rowsel", [4, 4, 128], BF16)
    e4t = load("c_e4t", [4, 128], F32)

    # persistent per-batch outputs
    srcTb = [big.tile([E, T], BF16, tag=f"srcT{b}", name=f"srcT{b}") for b in range(BPC)]
    h1b = [big.tile([E, T], BF16, tag=f"h1{b}", name=f"h1{b}") for b in range(BPC)]
    memb = [big.tile([E, T], BF16, tag=f"mem{b}", name=f"memb{b}") for b in range(BPC)]

    # ---------------- encoder: per batch ----------------
    for b in range(BPC):
        with nc.named_scope(f"enc_b{b}"):
            # src natural [128, 8, 128] f32 (one DMA)
            xsrc = scr.tile([128, NKT, E], F32, tag="xsrc", bufs=2)
            nc.sync.dma_start(out=xsrc[:],
                              in_=tens["src"].ap()[b].rearrange("(c p) e -> p c e", p=128))
            # masks: 2 DMAs of [128, 4, 1024] i32, cast to fp8 on Pool
            m8 = scr.tile([128, NKT, T], FP8, tag="m8", bufs=1 if _dbg else 2)
            for quar in range(4):
                mi = scr.tile([128, 2, T], I32, tag="mi", bufs=2)
                nc.sync.dma_start(
                    out=mi[:],
                    in_=tens["enc_mask"].ap()[b].rearrange(
                        "(c p) t -> p c t", p=128)[:, 2 * quar:2 * (quar + 1), :])
                nc.gpsimd.tensor_copy(m8[:, 2 * quar:2 * (quar + 1), :], mi[:])

            # LN1 along free axis (per 128-token tile)
            ssum = scr.tile([128, NKT], F32, tag="ssum")
            ssq = scr.tile([128, NKT], F32, tag="ssq")
            sqscr = scr.tile([128, NKT, E], F32, tag="sqscr")
            nc.vector.tensor_reduce(ssum[:].rearrange("p (t o) -> p t o", o=1),
                                    xsrc[:], axis=mybir.AxisListType.X, op=OP.add)
            nc.vector.tensor_mul(sqscr[:], xsrc[:], xsrc[:])
            nc.vector.tensor_reduce(ssq[:].rearrange("p (t o) -> p t o", o=1),
                                    sqscr[:], axis=mybir.AxisListType.X, op=OP.add)
            mcol = scr.tile([128, NKT], F32, tag="mcol")
            nc.vector.tensor_scalar_mul(mcol[:], ssum[:], 1.0 / E)
            var = scr.tile([128, NKT], F32, tag="var")
            nc.vector.scalar_tensor_tensor(out=var[:], in0=ssq[:], scalar=1.0 / E,
                                           in1=mcol[:], op0=OP.mult, op1=OP.bypass)
            msq = scr.tile([128, NKT], F32, tag="msq")
            nc.vector.tensor_mul(msq[:], mcol[:], mcol[:])
            nc.vector.tensor_sub(var[:], var[:], msq[:])
            sd = scr.tile([128, NKT], F32, tag="sd")
            nc.scalar.activation(out=sd[:], in_=var[:], func=AF.Sqrt, bias=eps128[:, 0:1])
            rs = scr.tile([128, NKT], F32, tag="rs")
            nc.vector.reciprocal(rs[:], sd[:])
            # xln bf16 tiles + PE transpose into XLT (bf16 psum), srcT (bf16)
            xln = scr.tile([128, NKT, E], BF16, tag="xln")
            for t in range(NKT):
                nc.vector.tensor_scalar(out=xln[:, t, :], in0=xsrc[:, t, :],
                                        scalar1=mcol[:, t:t + 1], scalar2=rs[:, t:t + 1],
                                        op0=OP.subtract, op1=OP.mult)
            xlt_ps = ps_T().bitcast(BF16)
            for t in range(NKT):
                nc.tensor.matmul(xlt_ps[:, 128 * t:128 * (t + 1)], xln[:, t, :],
                                 ident[:], start=True, stop=True, is_transpose=True)
            xlt8 = scr.tile([E, T], FP8, tag="xlt8")
            nc.vector.tensor_copy(xlt8[:], xlt_ps[:, 0:T])
            # src transpose for residual (bf16)
            xsb = scr.tile([128, NKT, E], BF16, tag="xsb")
            nc.vector.tensor_copy(xsb[:], xsrc[:])
            st_ps = ps_T().bitcast(BF16)
            for t in range(NKT):
                nc.tensor.matmul(st_ps[:, 128 * t:128 * (t + 1)], xsb[:, t, :],
                                 ident[:], start=True, stop=True, is_transpose=True)
            nc.vector.tensor_copy(srcTb[b][:], st_ps[:, 0:T])

            # ---- QKV (head-pair split so lhsT/rhs bases land on 0/32) ----
            # q8z: [64, hp 2, qp 4, i 2, 256] zero-interleaved, SC prefolded
            q8z = scr.tile([64, 2, 4, 2, 256], FP8, tag="q8z")
            nc.vector.memset(q8z[:], 0.0)
            # k8h: [64, hp 2, T]
            k8h = scr.tile([64, 2, T], FP8, tag="k8h")
            for c in range(2):
                s = slice(512 * c, 512 * (c + 1))
                for hp in range(2):
                    hs = slice(64 * hp, 64 * (hp + 1))
                    qp = ps_G()
                    nc.tensor.matmul(qp[0:64, :], w["enc_wq8"][:, hs], xlt8[:, s],
                                     start=True, stop=True)
                    base = q8z[:].rearrange("p a b i f -> p (a b i f)")
                    dst = bass.AP(tensor=base.tensor,
                                  offset=base.offset + 2048 * hp + 1024 * c,
                                  ap=[base.ap[0], [512, 2], [384, 2], [1, 128]])
                    nc.vector.tensor_scalar_mul(
                        dst, qp[0:64, :].rearrange("p (a i f) -> p a i f", a=2, i=2), SC)
                    kp_ = ps_G()
                    nc.tensor.matmul(kp_[0:64, :], w["enc_wk8"][:, hs], xlt8[:, s],
                                     start=True, stop=True)
                    nc.vector.tensor_copy(k8h[:, hp, s], kp_[0:64, :])
            # V natural (+ ones cols prefilled)
            v8 = scr.tile([128, NKP, 2, H, 32], FP8, tag="v8")
            vp = ps_S()
            for t in range(NKT):
                nc.tensor.matmul(vp[:, 128 * t:128 * (t + 1)],
                                 xlt8[:, 128 * t:128 * (t + 1)], w["enc_wv8"][:],
                                 start=True, stop=True)
            nc.vector.tensor_copy(
                v8[:].rearrange("p kp i h d -> p (kp i h d)"), vp[:])

            if _kph < 2:
                continue
            # ---- scores + mask + exp ----
            eu8 = scr.tile([128, NKP, 2, H, T], FP8, tag="eu8", bufs=1)
            for h in range(H):
                hb = slice(32 * (h % 2), 32 * (h % 2) + 32)
                krow = k8h[:, h // 2, :]
                for kt in range(NKT):
                    Sp = ps_S()
                    for qp_i in range(4):
                        qs = slice(256 * qp_i, 256 * (qp_i + 1))
                        nc.tensor.matmul(Sp[:, qs],
                                         rep2(krow[hb, 128 * kt:128 * (kt + 1)]),
                                         q8z[hb, h // 2, qp_i, :, :],
                                         start=True, stop=False, perf_mode=PM.DoubleRow)
                        nc.tensor.matmul(Sp[:, qs],
                                         m8[:, 2 * qp_i:2 * qp_i + 2,
                                            128 * kt:128 * (kt + 1)],
                                         ipair[:], start=False, stop=True,
                                         perf_mode=PM.DoubleRow)
                    nc.scalar.activation(out=eu8[:, kt // 2, kt % 2, h, :], in_=Sp[:],
                                         func=AF.Exp, bias=negC[:, 0:1])
                    if b == 0 and h == 0 and kt == 0 and "dbg_S" in tens:
                        Ssb = scr.tile([128, T], F32, tag="dbg")
                        nc.vector.tensor_copy(Ssb[:], Sp[:])
                        nc.sync.dma_start(out=tens["dbg_S"].ap(), in_=Ssb[:])
            # NOTE: mask adds NMSK (not NMSK*SC) because q8z is pre-scaled by SC
            # and the exp has scale=1 -> exp(SC*K.Q + NMSK*m). exp(-30) ~ 1e-13.

            if _kph < 3:
                continue
            # ---- AV + denom + normalize + wo ----
            for c in range(2):
                s = slice(512 * c, 512 * (c + 1))
                h1p = ps_G()
                for h in range(H):
                    av = ps_a()
                    dn = ps_d()
                    for kp_i in range(NKP):
                        nc.tensor.matmul(av[:], v8[:, kp_i, :, h, :],
                                         eu8[:, kp_i, :, h, s],
                                         start=(kp_i == 0), stop=(kp_i == NKP - 1),
                                         perf_mode=PM.DoubleRow)
                        nc.tensor.matmul(dn[:], ones8[:],
                                         eu8[:, kp_i, :, h, s],
                                         start=(kp_i == 0), stop=(kp_i == NKP - 1),
                                         perf_mode=PM.DoubleRow)
                    rc = scr.tile([32, 512], F32, tag="rc")
                    nc.vector.reciprocal(rc[:], dn[:])
                    hn = scr.tile([32, 512], BF16, tag="hn")
                    nc.vector.tensor_tensor(out=hn[:], in0=av[:], in1=rc[:], op=OP.mult)
                    if b == 0 and h == 0 and c == 0 and "dbg_dn" in tens:
                        dsb = scr.tile([32, 512], F32, tag="dbg2")
                        nc.vector.tensor_copy(dsb[:], dn[:])
                        nc.sync.dma_start(out=tens["dbg_dn"].ap(), in_=dsb[:])
                        asb = scr.tile([32, 512], F32, tag="dbg2")
                        nc.vector.tensor_copy(asb[:], av[:])
                        nc.sync.dma_start(out=tens["dbg_av"].ap(), in_=asb[:])
                    nc.tensor.matmul(h1p[:], w["enc_wo16"][:, h, :], hn[:],
                                     start=(h == 0), stop=(h == H - 1))
                nc.vector.tensor_add(h1b[b][:, s], h1p[:], srcTb[b][:, s])
            if b == 0 and "dbg_h1" in tens:
                h1f = scr.tile([128, T], F32, tag="dbg")
                nc.vector.tensor_copy(h1f[:], h1b[0][:])
                nc.sync.dma_start(out=tens["dbg_h1"].ap(), in_=h1f[:])
                x8f = scr.tile([128, T], F32, tag="dbg")
                nc.vector.tensor_copy(x8f[:], xlt8[:])
                nc.sync.dma_start(out=tens["dbg_xlt"].ap(), in_=x8f[:])

    if _kph < 5:
        stub = scr.tile([BPC, T], F32, tag="lg_m", name="stub", bufs=1)
        hsrc = h1b[0] if _kph >= 3 else srcTb[0]
        nc.vector.tensor_copy(stub[:], hsrc[0:BPC, :])
        nc.sync.dma_start(out=tens["out"].ap().rearrange("b q t -> (b q) t"), in_=stub[:])
        return

    # ---------------- encoder LN2 + FFN (all batches) ----------------
    def ln_stats_group(xs, tagn):
        """Partition-axis LN core stats for 4 bf16 [E, T] tiles.
        Returns (mrow, nmrow, rsrow): [4, T] f32/bf16 tiles (per-batch rows)."""
        stats = ps_S()
        for b, xt in enumerate(xs):
            sq = scr.tile([128, T], BF16, tag="lg_sq", name=f"sq{tagn}", bufs=2)
            nc.vector.tensor_mul(sq[:], xt[:], xt[:])
            for c in range(2):
                s = slice(512 * c, 512 * (c + 1))
                nc.tensor.matmul(stats[0:36, s], ind8[:, 36 * b:36 * (b + 1)],
                                 xt[:, s], start=(b == 0), stop=False)
                nc.tensor.matmul(stats[0:36, s], ind8[:, 36 * (4 + b):36 * (5 + b)],
                                 sq[:, s], start=False,
                                 stop=(b == len(xs) - 1))
        m = scr.tile([4, T], F32, tag="lg_m", name=f"m{tagn}")
        nc.vector.tensor_scalar_mul(m[:], stats[0:4, :], 1.0 / E)
        var = scr.tile([4, T], F32, tag="lg_v", name=f"v{tagn}")
        nc.vector.tensor_scalar_mul(var[:], stats[32:36, :], 1.0 / E)
        msq = scr.tile([4, T], F32, tag="lg_ms", name=f"ms{tagn}")
        nc.vector.tensor_mul(msq[:], m[:], m[:])
        nc.vector.tensor_sub(var[:], var[:], msq[:])
        sd = scr.tile([4, T], F32, tag="lg_ms", name=f"sd{tagn}")
        nc.scalar.activation(out=sd[:], in_=var[:], func=AF.Sqrt, bias=eps4[:, 0:1])
        rsf = scr.tile([4, T], F32, tag="lg_v", name=f"rsf{tagn}")
        nc.vector.reciprocal(rsf[:], sd[:])
        rs = scr.tile([4, T], BF16, tag="lg_rs", name=f"rs{tagn}", bufs=2)
        nc.vector.tensor_copy(rs[:], rsf[:])
        nm = scr.tile([4, T], BF16, tag="lg_nm", name=f"nm{tagn}", bufs=2)
        nc.vector.scalar_tensor_tensor(out=nm[:], in0=m[:], scalar=-1.0,
                                       in1=rsf[:], op0=OP.mult, op1=OP.mult)
        return rs, nm

    def ln_apply(xt, rs, nm, b, c, out, out_slice=None):
        """out[:, s] = x*(ones x rs_b) + (ones x nm_b) for 512-chunk c."""
        s = slice(512 * c, 512 * (c + 1))
        a_ps = ps_G()
        nc.tensor.matmul(a_ps[:], rowsel[:, b, :], rs[0:4, s], start=True, stop=True)
        b_ps = ps_G()
        nc.tensor.matmul(b_ps[:], rowsel[:, b, :], nm[0:4, s], start=True, stop=True)
        tmp = scr.tile([128, 512], F32, tag="lntmp")
        nc.vector.scalar_tensor_tensor(out=tmp[:], in0=xt[:, s], scalar=1.0,
                                       in1=a_ps[:], op0=OP.bypass, op1=OP.mult)
        dst = out[:, s] if out_slice is None else out_slice
        nc.vector.scalar_tensor_tensor(out=dst, in0=tmp[:], scalar=1.0,
                                       in1=b_ps[:], op0=OP.bypass, op1=OP.add)

    with nc.named_scope("enc_ffn"):
        rs2, nm2 = ln_stats_group(h1b, "l2")
        for b in range(BPC):
            act8 = scr.tile([128, 2, 2, T], FP8, tag="act8")
            hln = scr.tile([E, T], BF16, tag="hln")
            for c in range(2):
                ln_apply(h1b[b], rs2, nm2, b, c, hln)
            for fc in range(4):
                for c in range(2):
                    s = slice(512 * c, 512 * (c + 1))
                    g = ps_G()
                    nc.tensor.matmul(g[:], w["enc_w116"][:, 128 * fc:128 * (fc + 1)],
                                     hln[:, s], start=True, stop=True)
                    nc.scalar.activation(out=act8[:, fc // 2, fc % 2, s], in_=g[:],
                                         func=AF.Relu, bias=w["enc_b1c"][:, fc:fc + 1])
            for c in range(2):
                s = slice(512 * c, 512 * (c + 1))
                mp = ps_G()
                for p in range(2):
                    nc.tensor.matmul(mp[:], w["enc_w28"][:, p, :, :],
                                     act8[:, p, :, s], start=(p == 0), stop=(p == 1),
                                     perf_mode=PM.DoubleRow)
                tmp2 = scr.tile([128, 512], F32, tag="ffntmp")
                nc.vector.tensor_scalar(out=tmp2[:], in0=mp[:],
                                        scalar1=w["enc_b2c"][:, 0:1], scalar2=None,
                                        op0=OP.add)
                nc.vector.tensor_add(memb[b][:, s], tmp2[:], h1b[b][:, s])

    if _kph < 6:
        stub = scr.tile([BPC, T], F32, tag="lg_m", name="stub", bufs=1)
        nc.vector.tensor_copy(stub[:], memb[0][0:BPC, :])
        nc.sync.dma_start(out=tens["out"].ap().rearrange("b q t -> (b q) t"), in_=stub[:])
        return

    # ---------------- decoder ----------------
    with nc.named_scope("decoder"):
        tgtT = singles.tile([E, BPC], F32, tag="tgtT")
        nc.sync.dma_start(out=tgtT[:], in_=tens["tgt"].ap().rearrange("b q e -> e (b q)"))
        dmi = scr.tile([BPC, T], I32, tag="dmi", bufs=1)
        nc.sync.dma_start(out=dmi[:], in_=tens["dec_mask"].ap().rearrange("b q t -> (b q) t"))
        dmf = singles.tile([BPC, T], BF16, tag="dmf")
        nc.vector.tensor_scalar_mul(dmf[:], dmi[:], -10000.0)
        dmf32 = singles.tile([BPC, T], F32, tag="dmf32")
        nc.vector.tensor_scalar_mul(dmf32[:], dmi[:], -10000.0)
        dwf = singles.tile([BPC, T], F32, tag="dwf")
        nc.vector.tensor_scalar(out=dwf[:], in0=dmi[:], scalar1=-1.0, scalar2=1.0,
                                op0=OP.mult, op1=OP.add)

        def ln_small(x, n, tagn):
            """Core LN over partitions for [E, n] f32 -> bf16 (g/b folded away)."""
            xbf = scr.tile([E, BPC], BF16, tag=f"dx{tagn}")
            nc.vector.tensor_copy(xbf[:, :n], x[:, :n])
            sq = scr.tile([E, BPC], BF16, tag=f"dq{tagn}")
            nc.vector.tensor_mul(sq[:, :n], xbf[:, :n], xbf[:, :n])
            stp = ps_G()
            nc.tensor.matmul(stp[0:33, :n], ind2[:, 0:33], xbf[:, :n], start=True, stop=False)
            nc.tensor.matmul(stp[0:33, :n], ind2[:, 33:66], sq[:, :n], start=False, stop=True)
            st = scr.tile([33, BPC], F32, tag=f"ds{tagn}")
            nc.vector.tensor_copy(st[:, :n], stp[0:33, :n])
            mn = scr.tile([1, BPC], F32, tag=f"dm{tagn}")
            nc.vector.tensor_scalar_mul(mn[:, :n], st[0:1, :n], 1.0 / E)
            msq = scr.tile([1, BPC], F32, tag=f"dmq{tagn}")
            nc.vector.tensor_mul(msq[:, :n], mn[:, :n], mn[:, :n])
            var = scr.tile([1, BPC], F32, tag=f"dv{tagn}")
            nc.vector.tensor_scalar_mul(var[:, :n], st[32:33, :n], 1.0 / E)
            nc.vector.tensor_sub(var[:, :n], var[:, :n], msq[:, :n])
            sdd = scr.tile([1, BPC], F32, tag=f"dsd{tagn}")
            nc.scalar.activation(out=sdd[:, :n], in_=var[:, :n], func=AF.Sqrt,
                                 bias=eps1[:, 0:1])
            rsf = scr.tile([1, BPC], F32, tag=f"drf{tagn}")
            nc.vector.reciprocal(rsf[:, :n], sdd[:, :n])
            rs = scr.tile([1, BPC], BF16, tag=f"dr{tagn}")
            nc.vector.tensor_copy(rs[:, :n], rsf[:, :n])
            nm = scr.tile([1, BPC], BF16, tag=f"dn{tagn}")
            nc.vector.scalar_tensor_tensor(out=nm[:, :n], in0=mn[:, :n], scalar=-1.0,
                                           in1=rsf[:, :n], op0=OP.mult, op1=OP.mult)
            a_ps = ps_G()
            nc.tensor.matmul(a_ps[:, 0:n], onesrow[:], rs[:, :n], start=True, stop=True)
            b_ps = ps_G()
            nc.tensor.matmul(b_ps[:, 0:n], onesrow[:], nm[:, :n], start=True, stop=True)
            tmp = scr.tile([E, BPC], F32, tag=f"dt{tagn}")
            nc.vector.scalar_tensor_tensor(out=tmp[:, :n], in0=x[:, :n], scalar=1.0,
                                           in1=a_ps[:, 0:n], op0=OP.bypass, op1=OP.mult)
            out = scr.tile([E, BPC], BF16, tag=f"do{tagn}")
            nc.vector.scalar_tensor_tensor(out=out[:, :n], in0=tmp[:, :n], scalar=1.0,
                                           in1=b_ps[:, 0:n], op0=OP.bypass, op1=OP.add)
            return out

        tln = ln_small(tgtT, BPC, "t")
        qd_ps = ps_G()
        nc.tensor.matmul(qd_ps[0:HD, 0:BPC], w["dec_wq16"][:], tln[:, :BPC],
                         start=True, stop=True)
        qdec = scr.tile([HD, BPC], BF16, tag="qdec")
        nc.vector.tensor_copy(qdec[:], qd_ps[0:HD, 0:BPC])

        rsm, nmm = ln_stats_group(memb, "lm")
        h1d = singles.tile([E, BPC], F32, tag="h1d")
        for b in range(BPC):
            mln = scr.tile([E, T], BF16, tag="mln")
            for c in range(2):
                ln_apply(memb[b], rsm, nmm, b, c, mln)
            kd = scr.tile([HD, T], BF16, tag="kd", bufs=1)
            for c in range(2):
                s = slice(512 * c, 512 * (c + 1))
                kp_ = ps_G()
                nc.tensor.matmul(kp_[:], w["dec_wk16"][:], mln[:, s], start=True, stop=True)
                nc.vector.tensor_copy(kd[:, s], kp_[:])
            vd = scr.tile([128, NKT, HD], BF16, tag="vd", bufs=1)
            vp2 = ps_S()
            for t in range(NKT):
                nc.tensor.matmul(vp2[:, 128 * t:128 * (t + 1)],
                                 mln[:, 128 * t:128 * (t + 1)], w["dec_wv16"][:],
                                 start=True, stop=True)
            nc.vector.tensor_copy(vd[:], vp2[:].rearrange("p (c e) -> p c e", c=NKT))
            qblk = scr.tile([HD, 4], BF16, tag="qblk")
            nc.vector.memset(qblk[:], 0.0)
            for h in range(H):
                nc.vector.tensor_copy(qblk[32 * h:32 * (h + 1), h:h + 1],
                                      qdec[32 * h:32 * (h + 1), b:b + 1])
            ud_ps = ps_G()
            for kt in range(NKT):
                cs = slice(4 * kt, 4 * (kt + 1))
                nc.tensor.matmul(ud_ps[:, cs], kd[:, 128 * kt:128 * (kt + 1)], qblk[:],
                                 start=True, stop=False)
                nc.tensor.matmul(ud_ps[:, cs], dmf[0:4, 128 * kt:128 * (kt + 1)],
                                 dsel[:, 4 * b:4 * (b + 1)], start=False, stop=True)
            eud = scr.tile([128, 4 * NKT], BF16, tag="eud")
            nc.scalar.activation(out=eud[:], in_=ud_ps[:, 0:4 * NKT], func=AF.Exp,
                                 scale=SC)
            d1_ps = ps_G()
            nc.tensor.matmul(d1_ps[0:32, 0:1], eud[:], ones_col[:], start=True, stop=True)
            d1 = scr.tile([32, 1], F32, tag="d1s")
            nc.vector.tensor_copy(d1[:], d1_ps[0:32, 0:1])
            d4_ps = ps_G()
            nc.tensor.matmul(d4_ps[0:4, 0:1], p32[:], d1[:], start=True, stop=True)
            rc4 = scr.tile([4, 1], F32, tag="rc4")
            nc.vector.reciprocal(rc4[:], d4_ps[0:4, 0:1])
            rb_ps = ps_G()
            nc.tensor.matmul(rb_ps[:, 0:1], e4t[:], rc4[:], start=True, stop=True)
            rb = scr.tile([128, 1], F32, tag="rb")
            nc.vector.tensor_copy(rb[:], rb_ps[:, 0:1])
            hd_ps = ps_G()
            for kt in range(NKT):
                nc.tensor.matmul(hd_ps[:, 0:4], vd[:, kt, :], eud[:, 4 * kt:4 * (kt + 1)],
                                 start=(kt == 0), stop=(kt == NKT - 1))
            hdec = scr.tile([HD, 1], BF16, tag="hdec")
            for h in range(H):
                nc.vector.tensor_copy(hdec[32 * h:32 * (h + 1), 0:1],
                                      hd_ps[32 * h:32 * (h + 1), h:h + 1])
            nc.vector.tensor_scalar_mul(hdec[:], hdec[:], rb[:, 0:1])
            ao_ps = ps_G()
            nc.tensor.matmul(ao_ps[:, 0:1], w["dec_wo16f"][:], hdec[:],
                             start=True, stop=True)
            nc.vector.tensor_add(h1d[:, b:b + 1], ao_ps[:, 0:1], tgtT[:, b:b + 1])

        hln2d = ln_small(h1d, BPC, "d2")
        dact_ps = ps_G()
        for fc in range(4):
            nc.tensor.matmul(dact_ps[:, 4 * fc:4 * (fc + 1)],
                             w["dec_w116"][:, 128 * fc:128 * (fc + 1)], hln2d[:, :BPC],
                             start=True, stop=True)
        dact = scr.tile([128, 2, 2, 4], FP8, tag="dact")
        for fc in range(4):
            nc.vector.tensor_scalar(out=dact[:, fc // 2, fc % 2, :],
                                    in0=dact_ps[:, 4 * fc:4 * (fc + 1)],
                                    scalar1=w["dec_b1c"][:, fc:fc + 1],
                                    scalar2=0.0, op0=OP.add, op1=OP.max)
        do_ps = ps_G()
        for p in range(2):
            nc.tensor.matmul(do_ps[:, 0:BPC], w["dec_w28"][:, p, :, :],
                             dact[:, p, :, :], start=(p == 0), stop=(p == 1),
                             perf_mode=PM.DoubleRow)
        decT = singles.tile([E, BPC], F32, tag="decT")
        tmp2 = scr.tile([E, BPC], F32, tag="dtmp2")
        nc.vector.tensor_scalar(out=tmp2[:], in0=do_ps[:, 0:BPC],
                                scalar1=w["dec_b2c"][:, 0:1], scalar2=None, op0=OP.add)
        nc.vector.tensor_add(decT[:], tmp2[:], h1d[:])

    if _kph < 7:
        stub = scr.tile([BPC, T], F32, tag="lg_m", name="stub", bufs=1)
        nc.vector.tensor_copy(stub[:], memb[0][0:BPC, :])
        nc.sync.dma_start(out=tens["out"].ap().rearrange("b q t -> (b q) t"), in_=stub[:])
        return

    # ---------------- pointer ----------------
    with nc.named_scope("pointer"):
        dec16 = scr.tile([E, BPC], BF16, tag="dec16")
        nc.vector.tensor_copy(dec16[:], decT[:])
        qp_ps = ps_G()
        nc.tensor.matmul(qp_ps[:, 0:BPC], ptrq[:], dec16[:], start=True, stop=True)
        qpi = scr.tile([E, 4 * BPC], BF16, tag="qpi")
        nc.vector.memset(qpi[:], 0.0)
        for b in range(BPC):
            nc.vector.tensor_copy(qpi[:, 5 * b:5 * b + 1], qp_ps[:, b:b + 1])
        up_ps = ps_S()
        for b in range(BPC):
            kp8 = scr.tile([E, T], BF16, tag="kp8", bufs=1)
            for c in range(2):
                s = slice(512 * c, 512 * (c + 1))
                kpc = ps_G()
                nc.tensor.matmul(kpc[:], ptrk[:], memb[b][:, s], start=True, stop=True)
                nc.vector.tensor_copy(kp8[:, s], kpc[:])
            for c in range(2):
                s = slice(512 * c, 512 * (c + 1))
                nc.tensor.matmul(up_ps[0:BPC, s], qpi[:, 4 * b:4 * (b + 1)], kp8[:, s],
                                 start=(b == 0), stop=(b == BPC - 1))
        # L = 10*tanh(U/sqrt(E)); masked -> *dwf + dmf32; log_softmax
        th = scr.tile([BPC, T], F32, tag="th", bufs=1)
        nc.scalar.activation(out=th[:], in_=up_ps[0:BPC, :], func=AF.Tanh,
                             scale=1.0 / math.sqrt(E))
        L = scr.tile([BPC, T], F32, tag="L", bufs=1)
        nc.vector.scalar_tensor_tensor(out=L[:], in0=th[:], scalar=10.0,
                                       in1=dwf[:], op0=OP.mult, op1=OP.mult)
        nc.vector.tensor_add(L[:], L[:], dmf32[:])
        et = scr.tile([BPC, T], F32, tag="lg_ms", name="et", bufs=1)
        se = scr.tile([BPC, 1], F32, tag="se")
        nc.scalar.activation(out=et[:], in_=L[:], func=AF.Exp, accum_out=se[:])
        lse = scr.tile([BPC, 1], F32, tag="lse")
        nc.scalar.activation(out=lse[:], in_=se[:], func=AF.Ln)
        res = scr.tile([BPC, T], F32, tag="lg_m", name="res", bufs=1)
        nc.vector.tensor_scalar(out=res[:], in0=L[:], scalar1=lse[:, 0:1], scalar2=None,
                                op0=OP.subtract)
        nc.sync.dma_start(out=tens["out"].ap().rearrange("b q t -> (b q) t"), in_=res[:])


def build():
    import contextlib
    global DERIVED_SPECS
    nc = bacc.Bacc()
    tens = {}
    tens["src"] = nc.dram_tensor("src", [BPC, T, E], F32, kind="ExternalInput")
    tens["tgt"] = nc.dram_tensor("tgt", [BPC, Q, E], F32, kind="ExternalInput")
    tens["enc_mask"] = nc.dram_tensor("enc_mask", [BPC, T, T], I32, kind="ExternalInput")
    tens["dec_mask"] = nc.dram_tensor("dec_mask", [BPC, Q, T], I32, kind="ExternalInput")
    for name, arr in DERIVED_SPECS.items():
        if arr.dtype == F8:
            dt_ = mybir.dt.uint8
        elif arr.dtype == BF:
            dt_ = mybir.dt.uint16
        else:
            dt_ = mybir.dt.from_np(arr.dtype)
        tens[name] = nc.dram_tensor(name, list(arr.shape), dt_, kind="ExternalInput")
    tens["out"] = nc.dram_tensor("out", [BPC, Q, T], F32, kind="ExternalOutput")
    import os
    if os.environ.get("KDBG"):
        for nm, shp in [("dbg_S", [128, T]), ("dbg_dn", [32, 512]),
                        ("dbg_av", [32, 512]), ("dbg_h1", [128, T]),
                        ("dbg_xlt", [128, T])]:
            tens[nm] = nc.dram_tensor(nm, shp, F32, kind="ExternalOutput")

    with tile.TileContext(nc) as tc:
        with contextlib.ExitStack() as ctx:
            _emit(nc, tc, tens, ctx)
    nc.finalize()
    return nc


_built = {}


def _get_nc():
    if "nc" not in _built:
        _built["nc"] = build()
    return _built["nc"]


def make_in_maps(inputs, derived):
    in_maps = []
    for c in range(NCORES):
        s = slice(BPC * c, BPC * (c + 1))
        m = {
            "src": np.ascontiguousarray(np.asarray(inputs["src"])[s]),
            "tgt": np.ascontiguousarray(np.asarray(inputs["tgt"])[s]),
            "enc_mask": np.ascontiguousarray(np.asarray(inputs["enc_mask"])[s]),
            "dec_mask": np.ascontiguousarray(np.asarray(inputs["dec_mask"])[s]),
        }
        m.update(_wire_dtypes(derived))
        in_maps.append(m)
    return in_maps


def _wire_dtypes(derived):
    """fp8/bf16 arrays cross PJRT as uint8/uint16 (axon transfer-safe)."""
    out = {}
    for k, v in derived.items():
        if v.dtype == F8:
            out[k] = v.view(np.uint8)
        elif v.dtype == BF:
            out[k] = v.view(np.uint16)
        else:
            out[k] = v
    return out


def kernel(**inputs):
    global DERIVED_SPECS
    derived = host_prep(inputs)
    if DERIVED_SPECS is None:
        DERIVED_SPECS = {k: v for k, v in derived.items()}
    nc = _get_nc()
    in_maps = make_in_maps(inputs, derived)
    res = run_bass_kernel_spmd(nc, in_maps, list(range(NCORES)))
    out = np.concatenate([res.results[c]["out"] for c in range(NCORES)], axis=0)
    return out.astype(np.float32)


# revision 6
# speedup vs baseline: 1.3413x; 1.1391x over previous
"""Trainium2 Bass kernel for nn_AttentionNet — v2.

Sharding: data-parallel over batch, 4 batches/core x 8 cores.

Encoder design (per batch):
  - src loaded NATURAL [tok, E] (fast DMA), LN1 computed along free axis on
    DVE (g/b folded into QKV weights host-side), then PE-transposed.
  - Scores in [keys, q] layout via fp8 DoubleRow matmuls; the -30*mask add
    rides the same PSUM accumulation as DoubleRow pairs (lhsT = mask slice,
    rhs = const [-30I|0 ; 0|-30I]) so masking costs no extra engine pass.
  - exp on ACT -> fp8 SBUF (eu8); AV + replicated-denominator matmuls in
    DoubleRow; softmax normalize = DVE reciprocal + multiply.
  - FFN w2 in DoubleRow fp8; LN2/dec-LN gains folded into w1 host-side.
Decoder + pointer: bf16, ACT Tanh; masked -10000 path kept exact in f32.
"""

import math

import numpy as np
import ml_dtypes

import concourse.bacc as bacc
import concourse.bass as bass
import concourse.tile as tile
from concourse import mybir
from concourse.bass_utils import run_bass_kernel_spmd

F32 = mybir.dt.float32
BF16 = mybir.dt.bfloat16
FP8 = mybir.dt.float8e4
I32 = mybir.dt.int32
AF = mybir.ActivationFunctionType
OP = mybir.AluOpType
PM = mybir.MatmulPerfMode

E, H, D, FF = 128, 4, 32, 512
HD = H * D
B, T, Q = 32, 1024, 1
NCORES = 8
BPC = B // NCORES
NKT = T // 128   # key tiles
NKP = NKT // 2   # key-tile pairs
SC = 1.0 / math.sqrt(D)
NMSK = -30.0     # mask additive constant (exp(-30*SC... no: exp(SC*(-30)) happens
                 # AFTER scale; we bake scale so exp sees SC*S - 30*SC*m... see below)

BF = ml_dtypes.bfloat16
F8 = ml_dtypes.float8_e4m3


def rep2(ap_):
    """Repeat a [P, F] AP twice along a new middle free dim via stride 0."""
    return bass.AP(tensor=ap_.tensor, offset=ap_.offset,
                   ap=[ap_.ap[0], [0, 2]] + list(ap_.ap[1:]))


def host_prep(inputs):
    """Fold LN gains/biases into weights; pre-cast/lay out weights."""
    inp = {k: np.asarray(v) for k, v in inputs.items()}
    d = {}
    for pfx in ("enc", "dec"):
        g1 = inp[f"{pfx}_ln1_g"].astype(np.float64)
        b1 = inp[f"{pfx}_ln1_b"].astype(np.float64)
        g2 = inp[f"{pfx}_ln2_g"].astype(np.float64)
        b2 = inp[f"{pfx}_ln2_b"].astype(np.float64)
        assert np.allclose(b1, 0) and np.allclose(b2, 0), "ln bias fold not implemented"
        for nm in ("wq", "wk", "wv"):
            w = inp[f"{pfx}_{nm}"].astype(np.float64)  # [H, E, D]
            w = w * g1[None, :, None]
            # -> [E, H*D]
            d[f"{pfx}_{nm}8"] = np.ascontiguousarray(
                w.transpose(1, 0, 2).reshape(E, HD)).astype(F8)
            d[f"{pfx}_{nm}16"] = np.ascontiguousarray(
                w.transpose(1, 0, 2).reshape(E, HD)).astype(BF)
        # wo: [H, D, E] -> [D, H, E] (lhsT tiles [32, h, E] at base 0)
        wo = inp[f"{pfx}_wo"].astype(np.float64).transpose(1, 0, 2)
        d[f"{pfx}_wo16"] = np.ascontiguousarray(wo).astype(BF)
        d[f"{pfx}_wo16f"] = np.ascontiguousarray(
            inp[f"{pfx}_wo"].astype(np.float64).reshape(HD, E)).astype(BF)
        # ffn w1 [E, FF] with g2 fold
        w1 = inp[f"{pfx}_ffn_w1"].astype(np.float64) * g2[:, None]
        d[f"{pfx}_w116"] = np.ascontiguousarray(w1).astype(BF)
        d[f"{pfx}_b1c"] = np.ascontiguousarray(
            inp[f"{pfx}_ffn_b1"].reshape(4, 128).T).astype(np.float32)  # [128, 4]
        # w2 [FF, E] -> DR-paired [128, pair 2, i 2, E]
        w2 = inp[f"{pfx}_ffn_w2"].astype(np.float64).reshape(2, 2, 128, E)
        d[f"{pfx}_w28"] = np.ascontiguousarray(w2.transpose(2, 0, 1, 3)).astype(F8)
        d[f"{pfx}_w216"] = np.ascontiguousarray(w2.transpose(2, 0, 1, 3)).astype(BF)
        d[f"{pfx}_b2c"] = np.ascontiguousarray(
            inp[f"{pfx}_ffn_b2"].reshape(E, 1)).astype(np.float32)
    d["ptr_wq16"] = inp["ptr_wq"].astype(BF)
    d["ptr_wk16"] = inp["ptr_wk"].astype(BF)

    # constants
    ipair = np.zeros((128, 2, 256), np.float32)
    for i in range(2):
        for r in range(128):
            ipair[r, i, 128 * i + r] = NMSK
    d["c_ipair"] = ipair.astype(F8)
    d["c_ident16"] = np.eye(128).astype(BF)
    d["c_ones8"] = np.ones((128, 2, 32), np.float32).astype(F8)
    d["c_onesrow"] = np.ones((1, 128), np.float32).astype(BF)
    # ind8 for partition-axis LN stats (batch b sums -> row b, sumsq -> 32+b)
    a = np.zeros((128, 8 * 36), np.float32)
    for b in range(4):
        a[:, 36 * b + b] = 1.0
        a[:, 36 * (4 + b) + 32 + b] = 1.0
    d["c_ind8"] = a.astype(BF)
    # ind2 for small partition-axis LN (sum -> row 0, sumsq -> row 32)
    a = np.zeros((128, 66), np.float32)
    a[:, 0] = 1.0
    a[:, 33 + 32] = 1.0
    d["c_ind2"] = a.astype(BF)
    d["c_eps4"] = np.full((4, 1), 1e-5, np.float32)
    d["c_eps128"] = np.full((128, 1), 1e-5, np.float32)
    d["c_negC"] = np.full((128, 1), -4.0, np.float32)
    d["c_eps1"] = np.full((1, 1), 1e-5, np.float32)
    # dsel[k, 4b+h] = 1 iff k == b  (decoder mask select)
    a = np.zeros((4, 16), np.float32)
    for b in range(BPC):
        a[b, 4 * b:4 * (b + 1)] = 1.0
    d["c_dsel"] = a.astype(BF)
    d["c_ones_col"] = np.ones((128, 1), np.float32).astype(BF)
    a = np.zeros((4, 4, 128), np.float32)
    for b in range(BPC):
        a[b, b, :] = 1.0
    d["c_rowsel"] = a.astype(BF)
    a = np.zeros((32, 4), np.float32)
    for i in range(32):
        a[i, i % 4] = 1.0
    d["c_p32"] = a.astype(np.float32)
    a = np.zeros((4, 128), np.float32)
    for h in range(H):
        a[h, 32 * h:32 * (h + 1)] = 1.0
    d["c_e4t"] = a.astype(np.float32)
    return d


DERIVED_SPECS = None  # filled on first host_prep


def _emit(nc, tc, tens, ctx):
    import os
    _dbg = bool(os.environ.get("KDBG"))
    _kph = int(os.environ.get("KPH", "9"))
    singles = ctx.enter_context(tc.tile_pool(name="singles", bufs=1))
    psum = ctx.enter_context(tc.tile_pool(name="psum", bufs=1, space="PSUM"))
    big = ctx.enter_context(tc.tile_pool(name="big", bufs=1))
    scr = ctx.enter_context(tc.tile_pool(name="scr", bufs=1))

    cnt = [0]

    def ps_S():  # [128, 1024] f32: scores / V-proj / srcT staging (2 banks)
        cnt[0] += 1
        return psum.tile([128, 1024], F32, tag="S", name=f"S{cnt[0]}", bufs=2)

    def ps_T():  # [128, 1024] f32 transposes (shares S tag)
        return ps_S()

    def ps_a():  # [32, 512] f32 AV (1 bank)
        cnt[0] += 1
        return psum.tile([32, 512], F32, tag="a", name=f"a{cnt[0]}", bufs=1)

    def ps_d():  # [32, 512] f32 denom (1 bank)
        cnt[0] += 1
        return psum.tile([32, 512], F32, tag="d", name=f"d{cnt[0]}", bufs=1)

    def ps_G():  # [128, 512] f32 general (1 bank)
        cnt[0] += 1
        return psum.tile([128, 512], F32, tag="G", name=f"G{cnt[0]}", bufs=2)

    def load(name, shape, dt, ap=None):
        tl = singles.tile(shape, dt, tag=name, name=name)
        src_ap = tens[name].ap() if ap is None else ap
        if dt in (FP8, BF16) and src_ap.dtype != dt:
            src_ap = src_ap.bitcast(dt)
        nc.sync.dma_start(out=tl[:], in_=src_ap)
        return tl

    # ---- early src DMAs (batches 0-1) so LN1 starts before weights finish ----
    xsrc_pre = {}
    for b in range(2):
        t_ = scr.tile([128, NKT, E], F32, tag="xsrc", name=f"xsrcp{b}", bufs=2)
        nc.sync.dma_start(out=t_[:],
                          in_=tens["src"].ap()[b].rearrange("(c p) e -> p c e", p=128))
        xsrc_pre[b] = t_

    # ---- early constants (LN1/transposes/exp need these first) ----
    ident = load("c_ident16", [128, 128], BF16)
    eps128 = load("c_eps128", [128, 1], F32)
    negC = load("c_negC", [128, 1], F32)
    m8_pre = {}

    # ---- weights / constants to SBUF ----
    w = {}
    for pfx in ("enc", "dec"):
        for nm in ("wq8", "wk8", "wv8"):
            w[f"{pfx}_{nm}"] = load(f"{pfx}_{nm}", [E, HD], FP8)
        for nm in ("wq16", "wk16", "wv16"):
            w[f"{pfx}_{nm}"] = load(f"{pfx}_{nm}", [E, HD], BF16)
        w[f"{pfx}_wo16"] = load(f"{pfx}_wo16", [D, H, E], BF16)
        w[f"{pfx}_wo16f"] = load(f"{pfx}_wo16f", [HD, E], BF16)
        w[f"{pfx}_w116"] = load(f"{pfx}_w116", [E, FF], BF16)
        w[f"{pfx}_b1c"] = load(f"{pfx}_b1c", [128, 4], F32)
        w[f"{pfx}_w28"] = load(f"{pfx}_w28", [128, 2, 2, E], FP8)
        w[f"{pfx}_w216"] = load(f"{pfx}_w216", [128, 2, 2, E], BF16)
        w[f"{pfx}_b2c"] = load(f"{pfx}_b2c", [E, 1], F32)
    ptrq = load("ptr_wq16", [E, E], BF16)
    ptrk = load("ptr_wk16", [E, E], BF16)
    ipair = load("c_ipair", [128, 2, 256], FP8)
    ones8 = load("c_ones8", [128, 2, 32], FP8)
    onesrow = load("c_onesrow", [1, 128], BF16)
    ind8 = load("c_ind8", [128, 288], BF16)
    ind2 = load("c_ind2", [128, 66], BF16)
    eps4 = load("c_eps4", [4, 1], F32)
    eps1 = load("c_eps1", [1, 1], F32)
    dsel = load("c_dsel", [4, 16], BF16)
    ones_col = load("c_ones_col", [128, 1], BF16)
    p32 = load("c_p32", [32, 4], F32)
    rowsel = load("c_rowsel", [4, 4, 128], BF16)
    e4t = load("c_e4t", [4, 128], F32)

    # persistent per-batch outputs
    srcTb = [big.tile([E, T], BF16, tag=f"srcT{b}", name=f"srcT{b}") for b in range(BPC)]
    h1b = [big.tile([E, T], BF16, tag=f"h1{b}", name=f"h1{b}") for b in range(BPC)]
    memb = [big.tile([E, T], BF16, tag=f"mem{b}", name=f"memb{b}") for b in range(BPC)]

    # ---------------- encoder: per batch ----------------
    for b in range(BPC):
        with nc.named_scope(f"enc_b{b}"):
            # src natural [128, 8, 128] f32 (one DMA; batches 0-1 preloaded)
            if b in xsrc_pre:
                xsrc = xsrc_pre.pop(b)
            else:
                xsrc = scr.tile([128, NKT, E], F32, tag="xsrc", bufs=2)
                nc.sync.dma_start(out=xsrc[:],
                                  in_=tens["src"].ap()[b].rearrange("(c p) e -> p c e", p=128))
            # masks: 2 DMAs of [128, 4, 1024] i32, cast to fp8 on Pool
            m8 = scr.tile([128, NKT, T], FP8, tag="m8", bufs=1 if _dbg else 2)
            for quar in range(4):
                mi = scr.tile([128, 2, T], I32, tag="mi", bufs=2)
                nc.sync.dma_start(
                    out=mi[:],
                    in_=tens["enc_mask"].ap()[b].rearrange(
                        "(c p) t -> p c t", p=128)[:, 2 * quar:2 * (quar + 1), :])
                nc.gpsimd.tensor_copy(m8[:, 2 * quar:2 * (quar + 1), :], mi[:])

            # LN1 along free axis (per 128-token tile)
            ssum = scr.tile([128, NKT], F32, tag="ssum")
            ssq = scr.tile([128, NKT], F32, tag="ssq")
            sqscr = scr.tile([128, NKT, E], F32, tag="sqscr")
            nc.vector.tensor_reduce(ssum[:].rearrange("p (t o) -> p t o", o=1),
                                    xsrc[:], axis=mybir.AxisListType.X, op=OP.add)
            nc.vector.tensor_mul(sqscr[:], xsrc[:], xsrc[:])
            nc.vector.tensor_reduce(ssq[:].rearrange("p (t o) -> p t o", o=1),
                                    sqscr[:], axis=mybir.AxisListType.X, op=OP.add)
            mcol = scr.tile([128, NKT], F32, tag="mcol")
            nc.vector.tensor_scalar_mul(mcol[:], ssum[:], 1.0 / E)
            var = scr.tile([128, NKT], F32, tag="var")
            nc.vector.scalar_tensor_tensor(out=var[:], in0=ssq[:], scalar=1.0 / E,
                                           in1=mcol[:], op0=OP.mult, op1=OP.bypass)
            msq = scr.tile([128, NKT], F32, tag="msq")
            nc.vector.tensor_mul(msq[:], mcol[:], mcol[:])
            nc.vector.tensor_sub(var[:], var[:], msq[:])
            sd = scr.tile([128, NKT], F32, tag="sd")
            nc.scalar.activation(out=sd[:], in_=var[:], func=AF.Sqrt, bias=eps128[:, 0:1])
            rs = scr.tile([128, NKT], F32, tag="rs")
            nc.vector.reciprocal(rs[:], sd[:])
            # xln bf16 tiles + PE transpose into XLT (bf16 psum), srcT (bf16)
            xln = scr.tile([128, NKT, E], BF16, tag="xln")
            for t in range(NKT):
                nc.vector.tensor_scalar(out=xln[:, t, :], in0=xsrc[:, t, :],
                                        scalar1=mcol[:, t:t + 1], scalar2=rs[:, t:t + 1],
                                        op0=OP.subtract, op1=OP.mult)
            xlt8 = scr.tile([E, T], FP8, tag="xlt8")
            for gh in range(2):
                gp = ps_G().bitcast(BF16)
                for t4 in range(4):
                    t = 4 * gh + t4
                    nc.tensor.matmul(gp[:, 128 * t4:128 * (t4 + 1)], xln[:, t, :],
                                     ident[:], start=True, stop=True, is_transpose=True)
                nc.vector.tensor_copy(xlt8[:, 512 * gh:512 * (gh + 1)], gp[:, 0:512])
            # src transpose for residual (bf16)
            xsb = scr.tile([128, NKT, E], BF16, tag="xsb")
            nc.vector.tensor_copy(xsb[:], xsrc[:])
            for gh in range(2):
                gp = ps_G().bitcast(BF16)
                for t4 in range(4):
                    t = 4 * gh + t4
                    nc.tensor.matmul(gp[:, 128 * t4:128 * (t4 + 1)], xsb[:, t, :],
                                     ident[:], start=True, stop=True, is_transpose=True)
                nc.vector.tensor_copy(srcTb[b][:, 512 * gh:512 * (gh + 1)], gp[:, 0:512])

            # ---- QKV (head-pair split so lhsT/rhs bases land on 0/32) ----
            # q8z: [64, hp 2, qp 4, i 2, 256] zero-interleaved, SC prefolded
            q8z = scr.tile([64, 2, 4, 2, 256], FP8, tag="q8z")
            nc.vector.memset(q8z[:], 0.0)
            # k8h: [64, hp 2, T]
            k8h = scr.tile([64, 2, T], FP8, tag="k8h")
            for c in range(2):
                s = slice(512 * c, 512 * (c + 1))
                for hp in range(2):
                    hs = slice(64 * hp, 64 * (hp + 1))
                    qp = ps_G()
                    nc.tensor.matmul(qp[0:64, :], w["enc_wq8"][:, hs], xlt8[:, s],
                                     start=True, stop=True)
                    base = q8z[:].rearrange("p a b i f -> p (a b i f)")
                    dst = bass.AP(tensor=base.tensor,
                                  offset=base.offset + 2048 * hp + 1024 * c,
                                  ap=[base.ap[0], [512, 2], [384, 2], [1, 128]])
                    nc.vector.tensor_scalar_mul(
                        dst, qp[0:64, :].rearrange("p (a i f) -> p a i f", a=2, i=2), SC)
                    kp_ = ps_G()
                    nc.tensor.matmul(kp_[0:64, :], w["enc_wk8"][:, hs], xlt8[:, s],
                                     start=True, stop=True)
                    nc.vector.tensor_copy(k8h[:, hp, s], kp_[0:64, :])
            # V natural (+ ones cols prefilled)
            v8 = scr.tile([128, NKP, 2, H, 32], FP8, tag="v8")
            v8flat = v8[:].rearrange("p kp i h d -> p (kp i h d)")
            for gh in range(2):
                vp = ps_G()
                for t4 in range(4):
                    t = 4 * gh + t4
                    nc.tensor.matmul(vp[:, 128 * t4:128 * (t4 + 1)],
                                     xlt8[:, 128 * t:128 * (t + 1)], w["enc_wv8"][:],
                                     start=True, stop=True)
                nc.vector.tensor_copy(v8flat[:, 512 * gh:512 * (gh + 1)], vp[:])

            if _kph < 2:
                continue
            # ---- scores + mask + exp ----
            eu8 = scr.tile([128, NKP, 2, H, T], FP8, tag="eu8", bufs=1)
            for h in range(H):
                hb = slice(32 * (h % 2), 32 * (h % 2) + 32)
                krow = k8h[:, h // 2, :]
                for kt in range(NKT):
                    Sp = ps_S()
                    for qp_i in range(4):
                        qs = slice(256 * qp_i, 256 * (qp_i + 1))
                        nc.tensor.matmul(Sp[:, qs],
                                         rep2(krow[hb, 128 * kt:128 * (kt + 1)]),
                                         q8z[hb, h // 2, qp_i, :, :],
                                         start=True, stop=False, perf_mode=PM.DoubleRow)
                        nc.tensor.matmul(Sp[:, qs],
                                         m8[:, 2 * qp_i:2 * qp_i + 2,
                                            128 * kt:128 * (kt + 1)],
                                         ipair[:], start=False, stop=True,
                                         perf_mode=PM.DoubleRow)
                    nc.scalar.activation(out=eu8[:, kt // 2, kt % 2, h, :], in_=Sp[:],
                                         func=AF.Exp, bias=negC[:, 0:1])
                    if b == 0 and h == 0 and kt == 0 and "dbg_S" in tens:
                        Ssb = scr.tile([128, T], F32, tag="dbg")
                        nc.vector.tensor_copy(Ssb[:], Sp[:])
                        nc.sync.dma_start(out=tens["dbg_S"].ap(), in_=Ssb[:])
            # NOTE: mask adds NMSK (not NMSK*SC) because q8z is pre-scaled by SC
            # and the exp has scale=1 -> exp(SC*K.Q + NMSK*m). exp(-30) ~ 1e-13.

            if _kph < 3:
                continue
            # ---- AV + denom + normalize + wo ----
            for c in range(2):
                s = slice(512 * c, 512 * (c + 1))
                h1p = ps_G()
                for h in range(H):
                    av = ps_a()
                    dn = ps_d()
                    for kp_i in range(NKP):
                        nc.tensor.matmul(av[:], v8[:, kp_i, :, h, :],
                                         eu8[:, kp_i, :, h, s],
                                         start=(kp_i == 0), stop=(kp_i == NKP - 1),
                                         perf_mode=PM.DoubleRow)
                        nc.tensor.matmul(dn[:], ones8[:],
                                         eu8[:, kp_i, :, h, s],
                                         start=(kp_i == 0), stop=(kp_i == NKP - 1),
                                         perf_mode=PM.DoubleRow)
                    rc = scr.tile([32, 512], F32, tag="rc")
                    nc.vector.reciprocal(rc[:], dn[:])
                    hn = scr.tile([32, 512], BF16, tag="hn")
                    nc.vector.tensor_tensor(out=hn[:], in0=av[:], in1=rc[:], op=OP.mult)
                    if b == 0 and h == 0 and c == 0 and "dbg_dn" in tens:
                        dsb = scr.tile([32, 512], F32, tag="dbg2")
                        nc.vector.tensor_copy(dsb[:], dn[:])
                        nc.sync.dma_start(out=tens["dbg_dn"].ap(), in_=dsb[:])
                        asb = scr.tile([32, 512], F32, tag="dbg2")
                        nc.vector.tensor_copy(asb[:], av[:])
                        nc.sync.dma_start(out=tens["dbg_av"].ap(), in_=asb[:])
                    nc.tensor.matmul(h1p[:], w["enc_wo16"][:, h, :], hn[:],
                                     start=(h == 0), stop=(h == H - 1))
                nc.vector.tensor_add(h1b[b][:, s], h1p[:], srcTb[b][:, s])
            if b == 0 and "dbg_h1" in tens:
                h1f = scr.tile([128, T], F32, tag="dbg")
                nc.vector.tensor_copy(h1f[:], h1b[0][:])
                nc.sync.dma_start(out=tens["dbg_h1"].ap(), in_=h1f[:])
                x8f = scr.tile([128, T], F32, tag="dbg")
                nc.vector.tensor_copy(x8f[:], xlt8[:])
                nc.sync.dma_start(out=tens["dbg_xlt"].ap(), in_=x8f[:])

    if _kph < 5:
        stub = scr.tile([BPC, T], F32, tag="lg_m", name="stub", bufs=1)
        hsrc = h1b[0] if _kph >= 3 else srcTb[0]
        nc.vector.tensor_copy(stub[:], hsrc[0:BPC, :])
        nc.sync.dma_start(out=tens["out"].ap().rearrange("b q t -> (b q) t"), in_=stub[:])
        return

    # ---------------- encoder LN2 + FFN (all batches) ----------------
    def ln_stats_group(xs, tagn):
        """Partition-axis LN core stats for 4 bf16 [E, T] tiles.
        Returns (mrow, nmrow, rsrow): [4, T] f32/bf16 tiles (per-batch rows)."""
        stats = ps_S()
        for b, xt in enumerate(xs):
            sq = scr.tile([128, T], BF16, tag="lg_sq", name=f"sq{tagn}", bufs=2)
            nc.vector.tensor_mul(sq[:], xt[:], xt[:])
            for c in range(2):
                s = slice(512 * c, 512 * (c + 1))
                nc.tensor.matmul(stats[0:36, s], ind8[:, 36 * b:36 * (b + 1)],
                                 xt[:, s], start=(b == 0), stop=False)
                nc.tensor.matmul(stats[0:36, s], ind8[:, 36 * (4 + b):36 * (5 + b)],
                                 sq[:, s], start=False,
                                 stop=(b == len(xs) - 1))
        m = scr.tile([4, T], F32, tag="lg_m", name=f"m{tagn}")
        nc.vector.tensor_scalar_mul(m[:], stats[0:4, :], 1.0 / E)
        var = scr.tile([4, T], F32, tag="lg_v", name=f"v{tagn}")
        nc.vector.tensor_scalar_mul(var[:], stats[32:36, :], 1.0 / E)
        msq = scr.tile([4, T], F32, tag="lg_ms", name=f"ms{tagn}")
        nc.vector.tensor_mul(msq[:], m[:], m[:])
        nc.vector.tensor_sub(var[:], var[:], msq[:])
        sd = scr.tile([4, T], F32, tag="lg_ms", name=f"sd{tagn}")
        nc.scalar.activation(out=sd[:], in_=var[:], func=AF.Sqrt, bias=eps4[:, 0:1])
        rsf = scr.tile([4, T], F32, tag="lg_v", name=f"rsf{tagn}")
        nc.vector.reciprocal(rsf[:], sd[:])
        rs = scr.tile([4, T], BF16, tag="lg_rs", name=f"rs{tagn}", bufs=2)
        nc.vector.tensor_copy(rs[:], rsf[:])
        nm = scr.tile([4, T], BF16, tag="lg_nm", name=f"nm{tagn}", bufs=2)
        nc.vector.scalar_tensor_tensor(out=nm[:], in0=m[:], scalar=-1.0,
                                       in1=rsf[:], op0=OP.mult, op1=OP.mult)
        return rs, nm

    def ln_apply(xt, rs, nm, b, c, out, out_slice=None):
        """out[:, s] = x*(ones x rs_b) + (ones x nm_b) for 512-chunk c."""
        s = slice(512 * c, 512 * (c + 1))
        a_ps = ps_G()
        nc.tensor.matmul(a_ps[:], rowsel[:, b, :], rs[0:4, s], start=True, stop=True)
        b_ps = ps_G()
        nc.tensor.matmul(b_ps[:], rowsel[:, b, :], nm[0:4, s], start=True, stop=True)
        tmp = scr.tile([128, 512], F32, tag="lntmp")
        nc.vector.scalar_tensor_tensor(out=tmp[:], in0=xt[:, s], scalar=1.0,
                                       in1=a_ps[:], op0=OP.bypass, op1=OP.mult)
        dst = out[:, s] if out_slice is None else out_slice
        nc.vector.scalar_tensor_tensor(out=dst, in0=tmp[:], scalar=1.0,
                                       in1=b_ps[:], op0=OP.bypass, op1=OP.add)

    with nc.named_scope("enc_ffn"):
        rs2, nm2 = ln_stats_group(h1b, "l2")
        for b in range(BPC):
            act8 = scr.tile([128, 2, 2, T], FP8, tag="act8")
            hln = scr.tile([E, T], BF16, tag="hln")
            for c in range(2):
                ln_apply(h1b[b], rs2, nm2, b, c, hln)
            for fc in range(4):
                for c in range(2):
                    s = slice(512 * c, 512 * (c + 1))
                    g = ps_G()
                    nc.tensor.matmul(g[:], w["enc_w116"][:, 128 * fc:128 * (fc + 1)],
                                     hln[:, s], start=True, stop=True)
                    nc.scalar.activation(out=act8[:, fc // 2, fc % 2, s], in_=g[:],
                                         func=AF.Relu, bias=w["enc_b1c"][:, fc:fc + 1])
            for c in range(2):
                s = slice(512 * c, 512 * (c + 1))
                mp = ps_G()
                for p in range(2):
                    nc.tensor.matmul(mp[:], w["enc_w28"][:, p, :, :],
                                     act8[:, p, :, s], start=(p == 0), stop=(p == 1),
                                     perf_mode=PM.DoubleRow)
                tmp2 = scr.tile([128, 512], F32, tag="ffntmp")
                nc.vector.tensor_scalar(out=tmp2[:], in0=mp[:],
                                        scalar1=w["enc_b2c"][:, 0:1], scalar2=None,
                                        op0=OP.add)
                nc.vector.tensor_add(memb[b][:, s], tmp2[:], h1b[b][:, s])

    if _kph < 6:
        stub = scr.tile([BPC, T], F32, tag="lg_m", name="stub", bufs=1)
        nc.vector.tensor_copy(stub[:], memb[0][0:BPC, :])
        nc.sync.dma_start(out=tens["out"].ap().rearrange("b q t -> (b q) t"), in_=stub[:])
        return

    # ---------------- decoder ----------------
    with nc.named_scope("decoder"):
        tgtT = singles.tile([E, BPC], F32, tag="tgtT")
        nc.sync.dma_start(out=tgtT[:], in_=tens["tgt"].ap().rearrange("b q e -> e (b q)"))
        dmi = scr.tile([BPC, T], I32, tag="dmi", bufs=1)
        nc.sync.dma_start(out=dmi[:], in_=tens["dec_mask"].ap().rearrange("b q t -> (b q) t"))
        dmf = singles.tile([BPC, T], BF16, tag="dmf")
        nc.vector.tensor_scalar_mul(dmf[:], dmi[:], -10000.0)
        dmf32 = singles.tile([BPC, T], F32, tag="dmf32")
        nc.vector.tensor_scalar_mul(dmf32[:], dmi[:], -10000.0)
        dwf = singles.tile([BPC, T], F32, tag="dwf")
        nc.vector.tensor_scalar(out=dwf[:], in0=dmi[:], scalar1=-1.0, scalar2=1.0,
                                op0=OP.mult, op1=OP.add)

        def ln_small(x, n, tagn):
            """Core LN over partitions for [E, n] f32 -> bf16 (g/b folded away)."""
            xbf = scr.tile([E, BPC], BF16, tag=f"dx{tagn}")
            nc.vector.tensor_copy(xbf[:, :n], x[:, :n])
            sq = scr.tile([E, BPC], BF16, tag=f"dq{tagn}")
            nc.vector.tensor_mul(sq[:, :n], xbf[:, :n], xbf[:, :n])
            stp = ps_G()
            nc.tensor.matmul(stp[0:33, :n], ind2[:, 0:33], xbf[:, :n], start=True, stop=False)
            nc.tensor.matmul(stp[0:33, :n], ind2[:, 33:66], sq[:, :n], start=False, stop=True)
            st = scr.tile([33, BPC], F32, tag=f"ds{tagn}")
            nc.vector.tensor_copy(st[:, :n], stp[0:33, :n])
            mn = scr.tile([1, BPC], F32, tag=f"dm{tagn}")
            nc.vector.tensor_scalar_mul(mn[:, :n], st[0:1, :n], 1.0 / E)
            msq = scr.tile([1, BPC], F32, tag=f"dmq{tagn}")
            nc.vector.tensor_mul(msq[:, :n], mn[:, :n], mn[:, :n])
            var = scr.tile([1, BPC], F32, tag=f"dv{tagn}")
            nc.vector.tensor_scalar_mul(var[:, :n], st[32:33, :n], 1.0 / E)
            nc.vector.tensor_sub(var[:, :n], var[:, :n], msq[:, :n])
            sdd = scr.tile([1, BPC], F32, tag=f"dsd{tagn}")
            nc.scalar.activation(out=sdd[:, :n], in_=var[:, :n], func=AF.Sqrt,
                                 bias=eps1[:, 0:1])
            rsf = scr.tile([1, BPC], F32, tag=f"drf{tagn}")
            nc.vector.reciprocal(rsf[:, :n], sdd[:, :n])
            rs = scr.tile([1, BPC], BF16, tag=f"dr{tagn}")
            nc.vector.tensor_copy(rs[:, :n], rsf[:, :n])
            nm = scr.tile([1, BPC], BF16, tag=f"dn{tagn}")
            nc.vector.scalar_tensor_tensor(out=nm[:, :n], in0=mn[:, :n], scalar=-1.0,
                                           in1=rsf[:, :n], op0=OP.mult, op1=OP.mult)
            a_ps = ps_G()
            nc.tensor.matmul(a_ps[:, 0:n], onesrow[:], rs[:, :n], start=True, stop=True)
            b_ps = ps_G()
            nc.tensor.matmul(b_ps[:, 0:n], onesrow[:], nm[:, :n], start=True, stop=True)
            tmp = scr.tile([E, BPC], F32, tag=f"dt{tagn}")
            nc.vector.scalar_tensor_tensor(out=tmp[:, :n], in0=x[:, :n], scalar=1.0,
                                           in1=a_ps[:, 0:n], op0=OP.bypass, op1=OP.mult)
            out = scr.tile([E, BPC], BF16, tag=f"do{tagn}")
            nc.vector.scalar_tensor_tensor(out=out[:, :n], in0=tmp[:, :n], scalar=1.0,
                                           in1=b_ps[:, 0:n], op0=OP.bypass, op1=OP.add)
            return out

        tln = ln_small(tgtT, BPC, "t")
        qd_ps = ps_G()
        nc.tensor.matmul(qd_ps[0:HD, 0:BPC], w["dec_wq16"][:], tln[:, :BPC],
                         start=True, stop=True)
        qdec = scr.tile([HD, BPC], BF16, tag="qdec")
        nc.vector.tensor_copy(qdec[:], qd_ps[0:HD, 0:BPC])

        rsm, nmm = ln_stats_group(memb, "lm")
        h1d = singles.tile([E, BPC], F32, tag="h1d")
        for b in range(BPC):
            mln = scr.tile([E, T], BF16, tag="mln")
            for c in range(2):
                ln_apply(memb[b], rsm, nmm, b, c, mln)
            kd = scr.tile([HD, T], BF16, tag="kd", bufs=1)
            for c in range(2):
                s = slice(512 * c, 512 * (c + 1))
                kp_ = ps_G()
                nc.tensor.matmul(kp_[:], w["dec_wk16"][:], mln[:, s], start=True, stop=True)
                nc.vector.tensor_copy(kd[:, s], kp_[:])
            vd = scr.tile([128, NKT, HD], BF16, tag="vd", bufs=1)
            vp2 = ps_S()
            for t in range(NKT):
                nc.tensor.matmul(vp2[:, 128 * t:128 * (t + 1)],
                                 mln[:, 128 * t:128 * (t + 1)], w["dec_wv16"][:],
                                 start=True, stop=True)
            nc.vector.tensor_copy(vd[:], vp2[:].rearrange("p (c e) -> p c e", c=NKT))
            qblk = scr.tile([HD, 4], BF16, tag="qblk")
            nc.vector.memset(qblk[:], 0.0)
            for h in range(H):
                nc.vector.tensor_copy(qblk[32 * h:32 * (h + 1), h:h + 1],
                                      qdec[32 * h:32 * (h + 1), b:b + 1])
            ud_ps = ps_G()
            for kt in range(NKT):
                cs = slice(4 * kt, 4 * (kt + 1))
                nc.tensor.matmul(ud_ps[:, cs], kd[:, 128 * kt:128 * (kt + 1)], qblk[:],
                                 start=True, stop=False)
                nc.tensor.matmul(ud_ps[:, cs], dmf[0:4, 128 * kt:128 * (kt + 1)],
                                 dsel[:, 4 * b:4 * (b + 1)], start=False, stop=True)
            eud = scr.tile([128, 4 * NKT], BF16, tag="eud")
            nc.scalar.activation(out=eud[:], in_=ud_ps[:, 0:4 * NKT], func=AF.Exp,
                                 scale=SC)
            d1_ps = ps_G()
            nc.tensor.matmul(d1_ps[0:32, 0:1], eud[:], ones_col[:], start=True, stop=True)
            d1 = scr.tile([32, 1], F32, tag="d1s")
            nc.vector.tensor_copy(d1[:], d1_ps[0:32, 0:1])
            d4_ps = ps_G()
            nc.tensor.matmul(d4_ps[0:4, 0:1], p32[:], d1[:], start=True, stop=True)
            rc4 = scr.tile([4, 1], F32, tag="rc4")
            nc.vector.reciprocal(rc4[:], d4_ps[0:4, 0:1])
            rb_ps = ps_G()
            nc.tensor.matmul(rb_ps[:, 0:1], e4t[:], rc4[:], start=True, stop=True)
            rb = scr.tile([128, 1], F32, tag="rb")
            nc.vector.tensor_copy(rb[:], rb_ps[:, 0:1])
            hd_ps = ps_G()
            for kt in range(NKT):
                nc.tensor.matmul(hd_ps[:, 0:4], vd[:, kt, :], eud[:, 4 * kt:4 * (kt + 1)],
                                 start=(kt == 0), stop=(kt == NKT - 1))
            hdec = scr.tile([HD, 1], BF16, tag="hdec")
            for h in range(H):
                nc.vector.tensor_copy(hdec[32 * h:32 * (h + 1), 0:1],
                                      hd_ps[32 * h:32 * (h + 1), h:h + 1])
            nc.vector.tensor_scalar_mul(hdec[:], hdec[:], rb[:, 0:1])
            ao_ps = ps_G()
            nc.tensor.matmul(ao_ps[:, 0:1], w["dec_wo16f"][:], hdec[:],
                             start=True, stop=True)
            nc.vector.tensor_add(h1d[:, b:b + 1], ao_ps[:, 0:1], tgtT[:, b:b + 1])

        hln2d = ln_small(h1d, BPC, "d2")
        dact_ps = ps_G()
        for fc in range(4):
            nc.tensor.matmul(dact_ps[:, 4 * fc:4 * (fc + 1)],
                             w["dec_w116"][:, 128 * fc:128 * (fc + 1)], hln2d[:, :BPC],
                             start=True, stop=True)
        dact = scr.tile([128, 2, 2, 4], FP8, tag="dact")
        for fc in range(4):
            nc.vector.tensor_scalar(out=dact[:, fc // 2, fc % 2, :],
                                    in0=dact_ps[:, 4 * fc:4 * (fc + 1)],
                                    scalar1=w["dec_b1c"][:, fc:fc + 1],
                                    scalar2=0.0, op0=OP.add, op1=OP.max)
        do_ps = ps_G()
        for p in range(2):
            nc.tensor.matmul(do_ps[:, 0:BPC], w["dec_w28"][:, p, :, :],
                             dact[:, p, :, :], start=(p == 0), stop=(p == 1),
                             perf_mode=PM.DoubleRow)
        decT = singles.tile([E, BPC], F32, tag="decT")
        tmp2 = scr.tile([E, BPC], F32, tag="dtmp2")
        nc.vector.tensor_scalar(out=tmp2[:], in0=do_ps[:, 0:BPC],
                                scalar1=w["dec_b2c"][:, 0:1], scalar2=None, op0=OP.add)
        nc.vector.tensor_add(decT[:], tmp2[:], h1d[:])

    if _kph < 7:
        stub = scr.tile([BPC, T], F32, tag="lg_m", name="stub", bufs=1)
        nc.vector.tensor_copy(stub[:], memb[0][0:BPC, :])
        nc.sync.dma_start(out=tens["out"].ap().rearrange("b q t -> (b q) t"), in_=stub[:])
        return

    # ---------------- pointer ----------------
    with nc.named_scope("pointer"):
        dec16 = scr.tile([E, BPC], BF16, tag="dec16")
        nc.vector.tensor_copy(dec16[:], decT[:])
        qp_ps = ps_G()
        nc.tensor.matmul(qp_ps[:, 0:BPC], ptrq[:], dec16[:], start=True, stop=True)
        qpi = scr.tile([E, 4 * BPC], BF16, tag="qpi")
        nc.vector.memset(qpi[:], 0.0)
        for b in range(BPC):
            nc.vector.tensor_copy(qpi[:, 5 * b:5 * b + 1], qp_ps[:, b:b + 1])
        up_ps = ps_S()
        for b in range(BPC):
            kp8 = scr.tile([E, T], BF16, tag="kp8", bufs=1)
            for c in range(2):
                s = slice(512 * c, 512 * (c + 1))
                kpc = ps_G()
                nc.tensor.matmul(kpc[:], ptrk[:], memb[b][:, s], start=True, stop=True)
                nc.vector.tensor_copy(kp8[:, s], kpc[:])
            for c in range(2):
                s = slice(512 * c, 512 * (c + 1))
                nc.tensor.matmul(up_ps[0:BPC, s], qpi[:, 4 * b:4 * (b + 1)], kp8[:, s],
                                 start=(b == 0), stop=(b == BPC - 1))
        # L = 10*tanh(U/sqrt(E)); masked -> *dwf + dmf32; log_softmax
        th = scr.tile([BPC, T], F32, tag="th", bufs=1)
        nc.scalar.activation(out=th[:], in_=up_ps[0:BPC, :], func=AF.Tanh,
                             scale=1.0 / math.sqrt(E))
        L = scr.tile([BPC, T], F32, tag="L", bufs=1)
        nc.vector.scalar_tensor_tensor(out=L[:], in0=th[:], scalar=10.0,
                                       in1=dwf[:], op0=OP.mult, op1=OP.mult)
        nc.vector.tensor_add(L[:], L[:], dmf32[:])
        et = scr.tile([BPC, T], F32, tag="lg_ms", name="et", bufs=1)
        se = scr.tile([BPC, 1], F32, tag="se")
        nc.scalar.activation(out=et[:], in_=L[:], func=AF.Exp, accum_out=se[:])
        lse = scr.tile([BPC, 1], F32, tag="lse")
        nc.scalar.activation(out=lse[:], in_=se[:], func=AF.Ln)
        res = scr.tile([BPC, T], F32, tag="lg_m", name="res", bufs=1)
        nc.vector.tensor_scalar(out=res[:], in0=L[:], scalar1=lse[:, 0:1], scalar2=None,
                                op0=OP.subtract)
        nc.sync.dma_start(out=tens["out"].ap().rearrange("b q t -> (b q) t"), in_=res[:])


def build():
    import contextlib
    global DERIVED_SPECS
    nc = bacc.Bacc()
    tens = {}
    tens["src"] = nc.dram_tensor("src", [BPC, T, E], F32, kind="ExternalInput")
    tens["tgt"] = nc.dram_tensor("tgt", [BPC, Q, E], F32, kind="ExternalInput")
    tens["enc_mask"] = nc.dram_tensor("enc_mask", [BPC, T, T], I32, kind="ExternalInput")
    tens["dec_mask"] = nc.dram_tensor("dec_mask", [BPC, Q, T], I32, kind="ExternalInput")
    for name, arr in DERIVED_SPECS.items():
        if arr.dtype == F8:
            dt_ = mybir.dt.uint8
        elif arr.dtype == BF:
            dt_ = mybir.dt.uint16
        else:
            dt_ = mybir.dt.from_np(arr.dtype)
        tens[name] = nc.dram_tensor(name, list(arr.shape), dt_, kind="ExternalInput")
    tens["out"] = nc.dram_tensor("out", [BPC, Q, T], F32, kind="ExternalOutput")
    import os
    if os.environ.get("KDBG"):
        for nm, shp in [("dbg_S", [128, T]), ("dbg_dn", [32, 512]),
                        ("dbg_av", [32, 512]), ("dbg_h1", [128, T]),
                        ("dbg_xlt", [128, T])]:
            tens[nm] = nc.dram_tensor(nm, shp, F32, kind="ExternalOutput")

    with tile.TileContext(nc) as tc:
        with contextlib.ExitStack() as ctx:
            _emit(nc, tc, tens, ctx)
    nc.finalize()
    return nc


_built = {}


def _get_nc():
    if "nc" not in _built:
        _built["nc"] = build()
    return _built["nc"]


def make_in_maps(inputs, derived):
    in_maps = []
    for c in range(NCORES):
        s = slice(BPC * c, BPC * (c + 1))
        m = {
            "src": np.ascontiguousarray(np.asarray(inputs["src"])[s]),
            "tgt": np.ascontiguousarray(np.asarray(inputs["tgt"])[s]),
            "enc_mask": np.ascontiguousarray(np.asarray(inputs["enc_mask"])[s]),
            "dec_mask": np.ascontiguousarray(np.asarray(inputs["dec_mask"])[s]),
        }
        m.update(_wire_dtypes(derived))
        in_maps.append(m)
    return in_maps


def _wire_dtypes(derived):
    """fp8/bf16 arrays cross PJRT as uint8/uint16 (axon transfer-safe)."""
    out = {}
    for k, v in derived.items():
        if v.dtype == F8:
            out[k] = v.view(np.uint8)
        elif v.dtype == BF:
            out[k] = v.view(np.uint16)
        else:
            out[k] = v
    return out


def kernel(**inputs):
    global DERIVED_SPECS
    derived = host_prep(inputs)
    if DERIVED_SPECS is None:
        DERIVED_SPECS = {k: v for k, v in derived.items()}
    nc = _get_nc()
    in_maps = make_in_maps(inputs, derived)
    res = run_bass_kernel_spmd(nc, in_maps, list(range(NCORES)))
    out = np.concatenate([res.results[c]["out"] for c in range(NCORES)], axis=0)
    return out.astype(np.float32)
